# revision 6
# baseline (speedup 1.0000x reference)
"""LRU (diagonal complex linear recurrence) Trainium2 Bass kernel, v10.

Math (per batch b, channel h, time t = 0..L-1):
    u_t   = delta * (x_t @ B_real + i * x_t @ B_img)
    h_t   = lam * h_{t-1} + u_t,   h_{-1} = h0,  lam = r e^{i theta}
    out_t = Re(h_t)

Structure — time-decimated (m=2) polar scan with the pair-combine
folded into the GEMM:
  E_j := h_{2j} obeys E_j = lam^2 E_{j-1} + u~_j with
  u~_j = lam*u_{2j-1} + u_{2j} = x_{2j-1} @ (lam*Bd) + x_{2j} @ Bd.
  So a GEMM over pairs (x_{2j-1}, x_{2j}) with weight sets
  {btr2,bti2} = lam*(btr+i*bti) and {btr,bti} yields u~ directly at
  half resolution. Polar trick on lam^2 = r^2 e^{i*2theta}:
  E_j = e^{i*2theta(j+1)} G_j,  G_j = r^2 G_{j-1} + e^{-i*2theta(j+1)} u~_j,
  G_{-1} = h0 / lam  (host-computed, fp32).
  Even outputs: out_{2j}  = C.Gr - S.Gi           (C,S = cos/sin 2theta(j+1))
  Odd  outputs: out_{2j+1} = P1.Gr - P2.Gi + Re(u_{2j+1})
                (P1,P2 = r*cos/sin(theta(2j+3)); Re(u_odd) from a third
                 partial GEMM x_{2j+1} @ btr).
  Scans run on DVE at half resolution (scan is the only serial resource).
  ALL elementwise ops run on DVE in fp16 2x mode; GpSimd left idle
  (SBUF port contention degrades DVE otherwise). PSUM->SBUF staging on
  ScalarE. Final even/odd combines run as identity matmuls on the PE.

v10 schedule (vs v9): weights packed pairwise and tables packed per-hg
into single DMAs; weights+tables dispatched on the Activation HWDGE
queue while x streams on the SP queue (parallel descriptor generation —
v9 serialized ~127 dispatches at ~0.65us each on SP, so the first DMA
byte moved at ~8.7us). Work units are interleaved (hg0,hg1) x (b0,b1)
x j-halves so the first GEMMs need only the first x quarter, and the
final stream is tapered (1024,512,512) to shorten the drain chain.

Sharding: batch-parallel over 8 cores (2 batch elements each), SPMD.
"""

from contextlib import ExitStack

import numpy as np

import concourse.bass as bass
import concourse.tile as tile
from concourse import bacc, mybir

B, L, F, H = 16, 4096, 512, 512
N_CORES = 8
B_LOC = B // N_CORES
HG = H // 128
FG = F // 128
J = L // 2            # half-res length
JOP = 2064            # odd stream padded (quarters of 528, 16-col overlap)
W = 1024              # elementwise tile width (j-cols)
PW = 512              # PSUM gemm tile width (j-cols)
FP32 = mybir.dt.float32
F16 = mybir.dt.float16
A = mybir.AluOpType


def build_program():
    nc = bacc.Bacc("TRN2", target_bir_lowering=False, debug=False,
                   enable_asserts=False, num_devices=1)

    xe_d = nc.dram_tensor("xe", [B_LOC, F, J], F16, kind="ExternalInput").ap()
    xo_d = nc.dram_tensor("xo", [B_LOC, F, JOP], F16, kind="ExternalInput").ap()
    # weight walls: [F, 2, H] = (btr2, btr) and (bti2, bti)
    wa_d = nc.dram_tensor("wall_a", [F, 2, H], F16, kind="ExternalInput").ap()
    wb_d = nc.dram_tensor("wall_b", [F, 2, H], F16, kind="ExternalInput").ap()
    # tables packed per hg: [HG, 128, 2, J] = (ctab, stab) and (p1tab, p2tab)
    cs_d = nc.dram_tensor("cs_tab", [HG, 128, 2, J], F16, kind="ExternalInput").ap()
    ps_d_t = nc.dram_tensor("ps_tab", [HG, 128, 2, J], F16, kind="ExternalInput").ap()
    r2_d = nc.dram_tensor("r2vec", [H], FP32, kind="ExternalInput").ap()
    gr0_d = nc.dram_tensor("ginr", [H], FP32, kind="ExternalInput").ap()
    gi0_d = nc.dram_tensor("gini", [H], FP32, kind="ExternalInput").ap()
    oute_d = nc.dram_tensor("oute", [B_LOC, H, J], F16, kind="ExternalOutput").ap()
    outo_d = nc.dram_tensor("outo", [B_LOC, H, J], F16, kind="ExternalOutput").ap()

    with tile.TileContext(nc) as tc, ExitStack() as ctx:
        singles = ctx.enter_context(tc.tile_pool(name="singles", bufs=1))
        xt_pool = ctx.enter_context(tc.tile_pool(name="xt", bufs=1))
        tab_pool = ctx.enter_context(tc.tile_pool(name="tabs", bufs=2))
        u_pool = ctx.enter_context(tc.tile_pool(name="u", bufs=2))
        work = ctx.enter_context(tc.tile_pool(name="work", bufs=2))
        opool = ctx.enter_context(tc.tile_pool(name="opool", bufs=2))
        gpool = ctx.enter_context(tc.tile_pool(name="gpool", bufs=3))
        rpool = ctx.enter_context(tc.tile_pool(name="res", bufs=3))
        ps_ab = ctx.enter_context(tc.tile_pool(name="ps_ab", bufs=1, space="PSUM"))
        ps_c = ctx.enter_context(tc.tile_pool(name="ps_c", bufs=4, space="PSUM"))
        ps_d = ctx.enter_context(tc.tile_pool(name="ps_d", bufs=2, space="PSUM"))

        # ---- weights on the scalar (Activation) HWDGE queue: 2 dispatches
        wall_a = singles.tile([128, FG, 2, H], F16)   # [., fg, 0,*]=btr2 [.,fg,1,*]=btr
        wall_b = singles.tile([128, FG, 2, H], F16)
        nc.scalar.dma_start(out=wall_a,
                            in_=wa_d.rearrange("(fg p) t h -> p fg t h", p=128))
        nc.scalar.dma_start(out=wall_b,
                            in_=wb_d.rearrange("(fg p) t h -> p fg t h", p=128))

        # ---- x streams on the SP queue: one dispatch per (b, quarter, stream)
        xte = [[xt_pool.tile([128, FG, 512], F16, tag=f"xte{b}_{q}",
                             name=f"xte{b}_{q}") for q in range(4)]
               for b in range(B_LOC)]
        xto = [[xt_pool.tile([128, FG, 528], F16, tag=f"xto{b}_{q}",
                             name=f"xto{b}_{q}") for q in range(4)]
               for b in range(B_LOC)]

        def load_quarter(eng, b, q):
            xe_r = xe_d[b].rearrange("(fg p) j -> p fg j", p=128)
            xo_r = xo_d[b].rearrange("(fg p) j -> p fg j", p=128)
            eng.dma_start(out=xte[b][q], in_=xe_r[:, :, 512 * q:512 * q + 512])
            eng.dma_start(out=xto[b][q], in_=xo_r[:, :, 512 * q:512 * q + 528])

        # b0 on the SP queue (early, feeds the first GEMMs); b1 rides the
        # scalar queue after the hg0/hg1 tables
        for q in range(4):
            load_quarter(nc.sync, 0, q)

        # ---- small vectors + identity
        r2_s = singles.tile([128, HG], FP32)
        gr0_s = singles.tile([128, HG], FP32)
        gi0_s = singles.tile([128, HG], FP32)
        nc.sync.dma_start(out=r2_s, in_=r2_d.rearrange("(hg p) -> p hg", p=128))
        nc.sync.dma_start(out=gr0_s, in_=gr0_d.rearrange("(hg p) -> p hg", p=128))
        nc.sync.dma_start(out=gi0_s, in_=gi0_d.rearrange("(hg p) -> p hg", p=128))
        from concourse.masks import make_identity
        ident = singles.tile([128, 128], F16)
        make_identity(nc, ident)
        nident = singles.tile([128, 128], F16)
        nc.vector.tensor_scalar(nident, ident, -1.0, None, op0=A.mult)

        r2bc = singles.tile([128, HG, W], FP32)
        nc.vector.memset(r2bc, 1.0)
        for hg in range(HG):
            nc.vector.tensor_scalar(r2bc[:, hg, :], r2bc[:, hg, :],
                                    r2_s[:, hg:hg + 1], None, op0=A.mult)

        # ---- tables: per-hg single dispatches on the scalar queue
        cs_t = {}
        ps_t = {}

        def load_tabs(hg):
            cs_t[hg] = tab_pool.tile([128, 2, J], F16, tag="cs",
                                     name=f"cs{hg}")
            ps_t[hg] = tab_pool.tile([128, 2, J], F16, tag="ps",
                                     name=f"ps{hg}")
            nc.scalar.dma_start(out=cs_t[hg], in_=cs_d[hg])
            nc.scalar.dma_start(out=ps_t[hg], in_=ps_d_t[hg])

        load_tabs(0)
        load_tabs(1)
        for q in range(4):
            load_quarter(nc.scalar, 1, q)

        # ---- work unit list: (hg, b, j0, w). hg0/hg1 interleaved first so
        # early GEMMs need only early x quarters; final stream tapered.
        units = [
            (0, 0, 0, 1024), (1, 0, 0, 1024), (0, 0, 1024, 1024), (1, 0, 1024, 1024),
            (0, 1, 0, 1024), (1, 1, 0, 1024), (0, 1, 1024, 1024), (1, 1, 1024, 1024),
            (2, 0, 0, 1024), (3, 0, 0, 1024), (2, 0, 1024, 1024), (3, 0, 1024, 1024),
            (2, 1, 0, 1024), (3, 1, 0, 1024), (2, 1, 1024, 1024),
            (3, 1, 1024, 512), (3, 1, 1536, 512),
        ]

        pending = []

        def flush_one(p):
            for pc, ssl in p["pcs"]:
                nc.tensor.matmul(pc, ident, p["o3"][:, ssl],
                                 start=False, stop=False)
                nc.tensor.matmul(pc, nident, p["o4"][:, ssl],
                                 start=False, stop=True)
                nc.scalar.copy(out=p["res_o"][:, ssl], in_=pc)
                pd = ps_d.tile([128, PW], FP32, tag="pd")
                nc.tensor.matmul(pd, ident, p["o1"][:, ssl],
                                 start=True, stop=False)
                nc.tensor.matmul(pd, nident, p["o2"][:, ssl],
                                 start=False, stop=True)
                nc.scalar.copy(out=p["res_e"][:, ssl], in_=pd)
            nc.gpsimd.dma_start(out=oute_d[p["b"], p["hsl"], p["jsl"]],
                                in_=p["res_e"][:, :p["w"]])
            nc.gpsimd.dma_start(out=outo_d[p["b"], p["hsl"], p["jsl"]],
                                in_=p["res_o"][:, :p["w"]])

        gprev = {}

        for ui, (hg, b, j0, w) in enumerate(units):
            if ui == 8:
                load_tabs(2)
            if ui == 9:
                load_tabs(3)
            hsl = slice(hg * 128, (hg + 1) * 128)
            if len(pending) >= 2:
                flush_one(pending.pop(0))
            ur_sb = u_pool.tile([128, W], F16, tag="ur_sb")
            ui_sb = u_pool.tile([128, W], F16, tag="ui_sb")
            pcs = []
            for ps in range(w // PW):
                p0 = j0 + ps * PW
                q = p0 // 512                 # x quarter
                osl = slice(0, PW)
                osl1 = slice(1, PW + 1)
                ssl = slice(ps * PW, (ps + 1) * PW)
                pa = ps_ab.tile([128, PW], FP32, tag="pa")
                pb = ps_ab.tile([128, PW], FP32, tag="pb")
                pc = ps_c.tile([128, PW], FP32, tag="pc")
                for fg in range(FG):
                    nc.tensor.matmul(pa, wall_a[:, fg, 0, hsl],
                                     xto[b][q][:, fg, osl],
                                     start=(fg == 0), stop=False)
                for fg in range(FG):
                    nc.tensor.matmul(pa, wall_a[:, fg, 1, hsl],
                                     xte[b][q][:, fg, osl],
                                     start=False, stop=(fg == FG - 1))
                for fg in range(FG):
                    nc.tensor.matmul(pb, wall_b[:, fg, 0, hsl],
                                     xto[b][q][:, fg, osl],
                                     start=(fg == 0), stop=False)
                for fg in range(FG):
                    nc.tensor.matmul(pb, wall_b[:, fg, 1, hsl],
                                     xte[b][q][:, fg, osl],
                                     start=False, stop=(fg == FG - 1))
                for fg in range(FG):
                    nc.tensor.matmul(pc, wall_a[:, fg, 1, hsl],
                                     xto[b][q][:, fg, osl1],
                                     start=(fg == 0), stop=False)
                nc.scalar.copy(out=ur_sb[:, ssl], in_=pa)
                nc.scalar.copy(out=ui_sb[:, ssl], in_=pb)
                pcs.append((pc, ssl))

            jsl = slice(j0, j0 + w)
            cw = cs_t[hg][:, 0, jsl]
            sw = cs_t[hg][:, 1, jsl]
            # input rotation: v = e^{-i*2theta(j+1)} u~
            t1 = work.tile([128, W], F16, tag="t1")
            t2 = work.tile([128, W], F16, tag="t2")
            t3 = work.tile([128, W], F16, tag="t3")
            t4 = work.tile([128, W], F16, tag="t4")
            vr = work.tile([128, W], F16, tag="vr")
            vi = work.tile([128, W], F16, tag="vi")
            nc.vector.tensor_mul(t1[:, :w], cw, ur_sb[:, :w])
            nc.vector.tensor_mul(t2[:, :w], sw, ui_sb[:, :w])
            nc.vector.tensor_add(vr[:, :w], t1[:, :w], t2[:, :w])
            nc.vector.tensor_mul(t3[:, :w], cw, ui_sb[:, :w])
            nc.vector.tensor_mul(t4[:, :w], sw, ur_sb[:, :w])
            nc.vector.tensor_sub(vi[:, :w], t3[:, :w], t4[:, :w])

            gr = gpool.tile([128, W], F16, tag="gr")
            gi = gpool.tile([128, W], F16, tag="gi")
            if j0 == 0:
                init_r = gr0_s[:, hg:hg + 1]
                init_i = gi0_s[:, hg:hg + 1]
            else:
                gr_p, gi_p, wp = gprev[(hg, b)]
                init_r = gr_p[:, wp - 1:wp]
                init_i = gi_p[:, wp - 1:wp]
            nc.vector.tensor_tensor_scan(gr[:, :w], r2bc[:, hg, :w],
                                         vr[:, :w], init_r,
                                         op0=A.mult, op1=A.add)
            nc.vector.tensor_tensor_scan(gi[:, :w], r2bc[:, hg, :w],
                                         vi[:, :w], init_i,
                                         op0=A.mult, op1=A.add)
            gprev[(hg, b)] = (gr, gi, w)

            # output: even t=2j -> C.Gr - S.Gi ; odd -> P1.Gr - P2.Gi + uo
            o1 = opool.tile([128, W], F16, tag="o1")
            o2 = opool.tile([128, W], F16, tag="o2")
            o3 = opool.tile([128, W], F16, tag="o3")
            o4 = opool.tile([128, W], F16, tag="o4")
            res_e = rpool.tile([128, W], F16, tag="res_e")
            res_o = rpool.tile([128, W], F16, tag="res_o")
            nc.vector.tensor_mul(o1[:, :w], cw, gr[:, :w])
            nc.vector.tensor_mul(o2[:, :w], sw, gi[:, :w])
            nc.vector.tensor_mul(o3[:, :w], ps_t[hg][:, 0, jsl], gr[:, :w])
            nc.vector.tensor_mul(o4[:, :w], ps_t[hg][:, 1, jsl], gi[:, :w])
            pending.append(dict(pcs=pcs, o1=o1, o2=o2, o3=o3, o4=o4,
                                res_e=res_e, res_o=res_o,
                                b=b, hsl=hsl, jsl=jsl, w=w))

        while pending:
            flush_one(pending.pop(0))

    nc.compile()
    return nc


def _prepare(inputs):
    x = np.asarray(inputs["x"], dtype=np.float32)
    B_real = np.asarray(inputs["B_real"], dtype=np.float64)
    B_img = np.asarray(inputs["B_img"], dtype=np.float64)
    nu = np.asarray(inputs["nu"], dtype=np.float64)
    theta = np.asarray(inputs["theta"], dtype=np.float64)
    delta = np.asarray(inputs["delta"], dtype=np.float64)
    h0r = np.asarray(inputs["h0_real"], dtype=np.float64)
    h0i = np.asarray(inputs["h0_img"], dtype=np.float64)

    r = np.exp(-np.exp(nu))
    btr = B_real * delta[None, :]
    bti = B_img * delta[None, :]
    rc = r * np.cos(theta)
    rs = r * np.sin(theta)
    btr2 = btr * rc[None, :] - bti * rs[None, :]
    bti2 = btr * rs[None, :] + bti * rc[None, :]

    jj = np.arange(J, dtype=np.float64)
    ang_c = theta[:, None] * (2.0 * jj + 2.0)[None, :]   # 2theta(j+1)
    ctab = np.cos(ang_c)
    stab = np.sin(ang_c)
    ang_p = theta[:, None] * (2.0 * jj + 3.0)[None, :]   # theta(2j+3)
    p1 = r[:, None] * np.cos(ang_p)
    p2 = r[:, None] * np.sin(ang_p)

    # G_{-1} = h0 / lam = h0 * e^{-i theta} / r
    ginr = (h0r * np.cos(theta) + h0i * np.sin(theta)) / r
    gini = (h0i * np.cos(theta) - h0r * np.sin(theta)) / r

    xh = x.astype(np.float16)
    # host-side transpose to [B, F, J] so device loads are contiguous
    xe = np.ascontiguousarray(xh[:, 0::2, :].transpose(0, 2, 1))   # (B, F, J)
    xo = np.zeros((B, F, JOP), dtype=np.float16)
    xo[:, :, 1:J + 1] = xh[:, 1::2, :].transpose(0, 2, 1)          # slot p = x_{2p-1}

    wall_a = np.stack([btr2, btr], axis=1).astype(np.float16)      # (F, 2, H)
    wall_b = np.stack([bti2, bti], axis=1).astype(np.float16)
    cth = ctab.reshape(HG, 128, J)
    sth = stab.reshape(HG, 128, J)
    cs_tab = np.stack([cth, sth], axis=2).astype(np.float16)       # (HG,128,2,J)
    p1h = p1.reshape(HG, 128, J)
    p2h = p2.reshape(HG, 128, J)
    ps_tab = np.stack([p1h, p2h], axis=2).astype(np.float16)

    return dict(
        wall_a=wall_a, wall_b=wall_b,
        r2vec=(r * r).astype(np.float32),
        cs_tab=cs_tab, ps_tab=ps_tab,
        ginr=ginr.astype(np.float32), gini=gini.astype(np.float32),
    ), xe, xo


_NC_CACHE = {}


def get_program():
    if "nc" not in _NC_CACHE:
        _NC_CACHE["nc"] = build_program()
    return _NC_CACHE["nc"]


def make_in_maps(inputs):
    shared, xe, xo = _prepare(inputs)
    return [dict(xe=np.ascontiguousarray(xe[c * B_LOC:(c + 1) * B_LOC]),
                 xo=np.ascontiguousarray(xo[c * B_LOC:(c + 1) * B_LOC]),
                 **shared)
            for c in range(N_CORES)]


def kernel(**inputs) -> np.ndarray:
    from concourse.bass_utils import run_bass_kernel_spmd

    nc = get_program()
    in_maps = make_in_maps(inputs)
    res = run_bass_kernel_spmd(nc, in_maps, list(range(N_CORES)))
    out = np.empty((B, L, H), dtype=np.float32)
    for c in range(N_CORES):
        sl = slice(c * B_LOC, (c + 1) * B_LOC)
        out[sl, 0::2, :] = res.results[c]["oute"].transpose(0, 2, 1)
        out[sl, 1::2, :] = res.results[c]["outo"].transpose(0, 2, 1)
    return out


# revision 7
# speedup vs baseline: 1.0186x; 1.0186x over previous
"""LRU (diagonal complex linear recurrence) Trainium2 Bass kernel, v10.

Math (per batch b, channel h, time t = 0..L-1):
    u_t   = delta * (x_t @ B_real + i * x_t @ B_img)
    h_t   = lam * h_{t-1} + u_t,   h_{-1} = h0,  lam = r e^{i theta}
    out_t = Re(h_t)

Structure — time-decimated (m=2) polar scan with the pair-combine
folded into the GEMM:
  E_j := h_{2j} obeys E_j = lam^2 E_{j-1} + u~_j with
  u~_j = lam*u_{2j-1} + u_{2j} = x_{2j-1} @ (lam*Bd) + x_{2j} @ Bd.
  So a GEMM over pairs (x_{2j-1}, x_{2j}) with weight sets
  {btr2,bti2} = lam*(btr+i*bti) and {btr,bti} yields u~ directly at
  half resolution. Polar trick on lam^2 = r^2 e^{i*2theta}:
  E_j = e^{i*2theta(j+1)} G_j,  G_j = r^2 G_{j-1} + e^{-i*2theta(j+1)} u~_j,
  G_{-1} = h0 / lam  (host-computed, fp32).
  Even outputs: out_{2j}  = C.Gr - S.Gi           (C,S = cos/sin 2theta(j+1))
  Odd  outputs: out_{2j+1} = P1.Gr - P2.Gi + Re(u_{2j+1})
                (P1,P2 = r*cos/sin(theta(2j+3)); Re(u_odd) from a third
                 partial GEMM x_{2j+1} @ btr).
  Scans run on DVE at half resolution (scan is the only serial resource).
  ALL elementwise ops run on DVE in fp16 2x mode; GpSimd left idle
  (SBUF port contention degrades DVE otherwise). PSUM->SBUF staging on
  ScalarE. Final even/odd combines run as identity matmuls on the PE.

v10 schedule (vs v9): weights packed pairwise and tables packed per-hg
into single DMAs; weights+tables dispatched on the Activation HWDGE
queue while x streams on the SP queue (parallel descriptor generation —
v9 serialized ~127 dispatches at ~0.65us each on SP, so the first DMA
byte moved at ~8.7us). Work units are interleaved (hg0,hg1) x (b0,b1)
x j-halves so the first GEMMs need only the first x quarter, and the
final stream is tapered (1024,512,512) to shorten the drain chain.

Sharding: batch-parallel over 8 cores (2 batch elements each), SPMD.
"""

from contextlib import ExitStack

import numpy as np

import concourse.bass as bass
import concourse.tile as tile
from concourse import bacc, mybir

B, L, F, H = 16, 4096, 512, 512
N_CORES = 8
B_LOC = B // N_CORES
HG = H // 128
FG = F // 128
J = L // 2            # half-res length
JOP = 2064            # odd stream padded (quarters of 528, 16-col overlap)
W = 1024              # elementwise tile width (j-cols)
PW = 512              # PSUM gemm tile width (j-cols)
FP32 = mybir.dt.float32
F16 = mybir.dt.float16
A = mybir.AluOpType


def build_program():
    nc = bacc.Bacc("TRN2", target_bir_lowering=False, debug=False,
                   enable_asserts=False, num_devices=1)

    xe_d = nc.dram_tensor("xe", [B_LOC, F, J], F16, kind="ExternalInput").ap()
    xo_d = nc.dram_tensor("xo", [B_LOC, F, JOP], F16, kind="ExternalInput").ap()
    # weight walls: [F, 2, H] = (btr2, btr) and (bti2, bti)
    wa_d = nc.dram_tensor("wall_a", [F, 2, H], F16, kind="ExternalInput").ap()
    wb_d = nc.dram_tensor("wall_b", [F, 2, H], F16, kind="ExternalInput").ap()
    # tables packed per hg: [HG, 128, 2, J] = (ctab, stab) and (p1tab, p2tab)
    cs_d = nc.dram_tensor("cs_tab", [HG, 128, 2, J], F16, kind="ExternalInput").ap()
    ps_d_t = nc.dram_tensor("ps_tab", [HG, 128, 2, J], F16, kind="ExternalInput").ap()
    r2_d = nc.dram_tensor("r2vec", [H], FP32, kind="ExternalInput").ap()
    gr0_d = nc.dram_tensor("ginr", [H], FP32, kind="ExternalInput").ap()
    gi0_d = nc.dram_tensor("gini", [H], FP32, kind="ExternalInput").ap()
    oute_d = nc.dram_tensor("oute", [B_LOC, H, J], F16, kind="ExternalOutput").ap()
    outo_d = nc.dram_tensor("outo", [B_LOC, H, J], F16, kind="ExternalOutput").ap()

    with tile.TileContext(nc) as tc, ExitStack() as ctx:
        singles = ctx.enter_context(tc.tile_pool(name="singles", bufs=1))
        xt_pool = ctx.enter_context(tc.tile_pool(name="xt", bufs=1))
        tab_pool = ctx.enter_context(tc.tile_pool(name="tabs", bufs=2))
        u_pool = ctx.enter_context(tc.tile_pool(name="u", bufs=2))
        work = ctx.enter_context(tc.tile_pool(name="work", bufs=2))
        opool = ctx.enter_context(tc.tile_pool(name="opool", bufs=2))
        gpool = ctx.enter_context(tc.tile_pool(name="gpool", bufs=3))
        rpool = ctx.enter_context(tc.tile_pool(name="res", bufs=3))
        ps_ab = ctx.enter_context(tc.tile_pool(name="ps_ab", bufs=1, space="PSUM"))
        ps_c = ctx.enter_context(tc.tile_pool(name="ps_c", bufs=4, space="PSUM"))
        ps_d = ctx.enter_context(tc.tile_pool(name="ps_d", bufs=2, space="PSUM"))

        # ---- weights: wall_a (pa/pc) heads the SP queue, wall_b (pb)
        # heads the scalar queue, so both land in parallel
        wall_a = singles.tile([128, FG, 2, H], F16)   # [., fg, 0,*]=btr2 [.,fg,1,*]=btr
        wall_b = singles.tile([128, FG, 2, H], F16)
        nc.sync.dma_start(out=wall_a,
                          in_=wa_d.rearrange("(fg p) t h -> p fg t h", p=128))
        nc.scalar.dma_start(out=wall_b,
                            in_=wb_d.rearrange("(fg p) t h -> p fg t h", p=128))

        # ---- x streams on the SP queue: one dispatch per (b, quarter, stream)
        xte = [[xt_pool.tile([128, FG, 512], F16, tag=f"xte{b}_{q}",
                             name=f"xte{b}_{q}") for q in range(4)]
               for b in range(B_LOC)]
        xto = [[xt_pool.tile([128, FG, 528], F16, tag=f"xto{b}_{q}",
                             name=f"xto{b}_{q}") for q in range(4)]
               for b in range(B_LOC)]

        def load_quarter(eng, b, q):
            xe_r = xe_d[b].rearrange("(fg p) j -> p fg j", p=128)
            xo_r = xo_d[b].rearrange("(fg p) j -> p fg j", p=128)
            eng.dma_start(out=xte[b][q], in_=xe_r[:, :, 512 * q:512 * q + 512])
            eng.dma_start(out=xto[b][q], in_=xo_r[:, :, 512 * q:512 * q + 528])

        # b0 on the SP queue (early, feeds the first GEMMs); b1 rides the
        # scalar queue after the hg0/hg1 tables
        for q in range(4):
            load_quarter(nc.sync, 0, q)

        # ---- small vectors + identity
        r2_s = singles.tile([128, HG], FP32)
        gr0_s = singles.tile([128, HG], FP32)
        gi0_s = singles.tile([128, HG], FP32)
        nc.sync.dma_start(out=r2_s, in_=r2_d.rearrange("(hg p) -> p hg", p=128))
        nc.sync.dma_start(out=gr0_s, in_=gr0_d.rearrange("(hg p) -> p hg", p=128))
        nc.sync.dma_start(out=gi0_s, in_=gi0_d.rearrange("(hg p) -> p hg", p=128))
        from concourse.masks import make_identity
        ident = singles.tile([128, 128], F16)
        make_identity(nc, ident)
        nident = singles.tile([128, 128], F16)
        nc.vector.tensor_scalar(nident, ident, -1.0, None, op0=A.mult)

        r2bc = singles.tile([128, HG, W], FP32)
        nc.vector.memset(r2bc, 1.0)
        for hg in range(HG):
            nc.vector.tensor_scalar(r2bc[:, hg, :], r2bc[:, hg, :],
                                    r2_s[:, hg:hg + 1], None, op0=A.mult)

        # ---- tables: per-hg single dispatches on the scalar queue
        cs_t = {}
        ps_t = {}

        def alloc_tabs(hg):
            cs_t[hg] = tab_pool.tile([128, 2, J], F16, tag="cs",
                                     name=f"cs{hg}")
            ps_t[hg] = tab_pool.tile([128, 2, J], F16, tag="ps",
                                     name=f"ps{hg}")

        def load_tab_half(hg, half):
            jsl = slice(half * J // 2, (half + 1) * J // 2)
            nc.scalar.dma_start(out=cs_t[hg][:, :, jsl], in_=cs_d[hg][:, :, jsl])
            nc.scalar.dma_start(out=ps_t[hg][:, :, jsl], in_=ps_d_t[hg][:, :, jsl])

        # scalar queue order mirrors first-need times: hg0/hg1 first halves,
        # then second halves, then b1's x, then the hg2/hg3 tables
        alloc_tabs(0)
        alloc_tabs(1)
        load_tab_half(0, 0)
        load_tab_half(1, 0)
        load_tab_half(0, 1)
        load_tab_half(1, 1)
        for q in range(4):
            load_quarter(nc.scalar, 1, q)
        alloc_tabs(2)
        alloc_tabs(3)
        load_tab_half(2, 0)
        load_tab_half(3, 0)
        load_tab_half(2, 1)
        load_tab_half(3, 1)

        # ---- work unit list: (hg, b, j0, w). hg0/hg1 interleaved first so
        # early GEMMs need only early x quarters; final stream tapered.
        units = [
            (0, 0, 0, 1024), (1, 0, 0, 1024), (0, 0, 1024, 1024), (1, 0, 1024, 1024),
            (0, 1, 0, 1024), (1, 1, 0, 1024), (0, 1, 1024, 1024), (1, 1, 1024, 1024),
            (2, 0, 0, 1024), (3, 0, 0, 1024), (2, 0, 1024, 1024), (3, 0, 1024, 1024),
            (2, 1, 0, 1024), (3, 1, 0, 1024), (2, 1, 1024, 1024),
            (3, 1, 1024, 512), (3, 1, 1536, 512),
        ]

        pending = []

        def flush_one(p):
            for pc, ssl in p["pcs"]:
                nc.tensor.matmul(pc, ident, p["o3"][:, ssl],
                                 start=False, stop=False)
                nc.tensor.matmul(pc, nident, p["o4"][:, ssl],
                                 start=False, stop=True)
                nc.scalar.copy(out=p["res_o"][:, ssl], in_=pc)
                pd = ps_d.tile([128, PW], FP32, tag="pd")
                nc.tensor.matmul(pd, ident, p["o1"][:, ssl],
                                 start=True, stop=False)
                nc.tensor.matmul(pd, nident, p["o2"][:, ssl],
                                 start=False, stop=True)
                nc.scalar.copy(out=p["res_e"][:, ssl], in_=pd)
            nc.sync.dma_start(out=oute_d[p["b"], p["hsl"], p["jsl"]],
                              in_=p["res_e"][:, :p["w"]])
            nc.sync.dma_start(out=outo_d[p["b"], p["hsl"], p["jsl"]],
                              in_=p["res_o"][:, :p["w"]])

        gprev = {}

        for ui, (hg, b, j0, w) in enumerate(units):
            hsl = slice(hg * 128, (hg + 1) * 128)
            if len(pending) >= 2:
                flush_one(pending.pop(0))
            ur_sb = u_pool.tile([128, W], F16, tag="ur_sb")
            ui_sb = u_pool.tile([128, W], F16, tag="ui_sb")
            pcs = []
            for ps in range(w // PW):
                p0 = j0 + ps * PW
                q = p0 // 512                 # x quarter
                osl = slice(0, PW)
                osl1 = slice(1, PW + 1)
                ssl = slice(ps * PW, (ps + 1) * PW)
                pa = ps_ab.tile([128, PW], FP32, tag="pa")
                pb = ps_ab.tile([128, PW], FP32, tag="pb")
                pc = ps_c.tile([128, PW], FP32, tag="pc")
                for fg in range(FG):
                    nc.tensor.matmul(pa, wall_a[:, fg, 0, hsl],
                                     xto[b][q][:, fg, osl],
                                     start=(fg == 0), stop=False)
                for fg in range(FG):
                    nc.tensor.matmul(pa, wall_a[:, fg, 1, hsl],
                                     xte[b][q][:, fg, osl],
                                     start=False, stop=(fg == FG - 1))
                for fg in range(FG):
                    nc.tensor.matmul(pb, wall_b[:, fg, 0, hsl],
                                     xto[b][q][:, fg, osl],
                                     start=(fg == 0), stop=False)
                for fg in range(FG):
                    nc.tensor.matmul(pb, wall_b[:, fg, 1, hsl],
                                     xte[b][q][:, fg, osl],
                                     start=False, stop=(fg == FG - 1))
                for fg in range(FG):
                    nc.tensor.matmul(pc, wall_a[:, fg, 1, hsl],
                                     xto[b][q][:, fg, osl1],
                                     start=(fg == 0), stop=False)
                nc.scalar.copy(out=ur_sb[:, ssl], in_=pa)
                nc.scalar.copy(out=ui_sb[:, ssl], in_=pb)
                pcs.append((pc, ssl))

            jsl = slice(j0, j0 + w)
            cw = cs_t[hg][:, 0, jsl]
            sw = cs_t[hg][:, 1, jsl]
            # input rotation: v = e^{-i*2theta(j+1)} u~
            t1 = work.tile([128, W], F16, tag="t1")
            t2 = work.tile([128, W], F16, tag="t2")
            t3 = work.tile([128, W], F16, tag="t3")
            t4 = work.tile([128, W], F16, tag="t4")
            vr = work.tile([128, W], F16, tag="vr")
            vi = work.tile([128, W], F16, tag="vi")
            nc.vector.tensor_mul(t1[:, :w], cw, ur_sb[:, :w])
            nc.vector.tensor_mul(t2[:, :w], sw, ui_sb[:, :w])
            nc.vector.tensor_add(vr[:, :w], t1[:, :w], t2[:, :w])
            nc.vector.tensor_mul(t3[:, :w], cw, ui_sb[:, :w])
            nc.vector.tensor_mul(t4[:, :w], sw, ur_sb[:, :w])
            nc.vector.tensor_sub(vi[:, :w], t3[:, :w], t4[:, :w])

            gr = gpool.tile([128, W], F16, tag="gr")
            gi = gpool.tile([128, W], F16, tag="gi")
            if j0 == 0:
                init_r = gr0_s[:, hg:hg + 1]
                init_i = gi0_s[:, hg:hg + 1]
            else:
                gr_p, gi_p, wp = gprev[(hg, b)]
                init_r = gr_p[:, wp - 1:wp]
                init_i = gi_p[:, wp - 1:wp]
            nc.vector.tensor_tensor_scan(gr[:, :w], r2bc[:, hg, :w],
                                         vr[:, :w], init_r,
                                         op0=A.mult, op1=A.add)
            nc.vector.tensor_tensor_scan(gi[:, :w], r2bc[:, hg, :w],
                                         vi[:, :w], init_i,
                                         op0=A.mult, op1=A.add)
            gprev[(hg, b)] = (gr, gi, w)

            # output: even t=2j -> C.Gr - S.Gi ; odd -> P1.Gr - P2.Gi + uo
            o1 = opool.tile([128, W], F16, tag="o1")
            o2 = opool.tile([128, W], F16, tag="o2")
            o3 = opool.tile([128, W], F16, tag="o3")
            o4 = opool.tile([128, W], F16, tag="o4")
            res_e = rpool.tile([128, W], F16, tag="res_e")
            res_o = rpool.tile([128, W], F16, tag="res_o")
            nc.vector.tensor_mul(o1[:, :w], cw, gr[:, :w])
            nc.vector.tensor_mul(o2[:, :w], sw, gi[:, :w])
            nc.vector.tensor_mul(o3[:, :w], ps_t[hg][:, 0, jsl], gr[:, :w])
            nc.vector.tensor_mul(o4[:, :w], ps_t[hg][:, 1, jsl], gi[:, :w])
            pending.append(dict(pcs=pcs, o1=o1, o2=o2, o3=o3, o4=o4,
                                res_e=res_e, res_o=res_o,
                                b=b, hsl=hsl, jsl=jsl, w=w))

        while pending:
            flush_one(pending.pop(0))

    nc.compile()
    return nc


def _prepare(inputs):
    x = np.asarray(inputs["x"], dtype=np.float32)
    B_real = np.asarray(inputs["B_real"], dtype=np.float64)
    B_img = np.asarray(inputs["B_img"], dtype=np.float64)
    nu = np.asarray(inputs["nu"], dtype=np.float64)
    theta = np.asarray(inputs["theta"], dtype=np.float64)
    delta = np.asarray(inputs["delta"], dtype=np.float64)
    h0r = np.asarray(inputs["h0_real"], dtype=np.float64)
    h0i = np.asarray(inputs["h0_img"], dtype=np.float64)

    r = np.exp(-np.exp(nu))
    btr = B_real * delta[None, :]
    bti = B_img * delta[None, :]
    rc = r * np.cos(theta)
    rs = r * np.sin(theta)
    btr2 = btr * rc[None, :] - bti * rs[None, :]
    bti2 = btr * rs[None, :] + bti * rc[None, :]

    jj = np.arange(J, dtype=np.float64)
    ang_c = theta[:, None] * (2.0 * jj + 2.0)[None, :]   # 2theta(j+1)
    ctab = np.cos(ang_c)
    stab = np.sin(ang_c)
    ang_p = theta[:, None] * (2.0 * jj + 3.0)[None, :]   # theta(2j+3)
    p1 = r[:, None] * np.cos(ang_p)
    p2 = r[:, None] * np.sin(ang_p)

    # G_{-1} = h0 / lam = h0 * e^{-i theta} / r
    ginr = (h0r * np.cos(theta) + h0i * np.sin(theta)) / r
    gini = (h0i * np.cos(theta) - h0r * np.sin(theta)) / r

    xh = x.astype(np.float16)
    # host-side transpose to [B, F, J] so device loads are contiguous
    xe = np.ascontiguousarray(xh[:, 0::2, :].transpose(0, 2, 1))   # (B, F, J)
    xo = np.zeros((B, F, JOP), dtype=np.float16)
    xo[:, :, 1:J + 1] = xh[:, 1::2, :].transpose(0, 2, 1)          # slot p = x_{2p-1}

    wall_a = np.stack([btr2, btr], axis=1).astype(np.float16)      # (F, 2, H)
    wall_b = np.stack([bti2, bti], axis=1).astype(np.float16)
    cth = ctab.reshape(HG, 128, J)
    sth = stab.reshape(HG, 128, J)
    cs_tab = np.stack([cth, sth], axis=2).astype(np.float16)       # (HG,128,2,J)
    p1h = p1.reshape(HG, 128, J)
    p2h = p2.reshape(HG, 128, J)
    ps_tab = np.stack([p1h, p2h], axis=2).astype(np.float16)

    return dict(
        wall_a=wall_a, wall_b=wall_b,
        r2vec=(r * r).astype(np.float32),
        cs_tab=cs_tab, ps_tab=ps_tab,
        ginr=ginr.astype(np.float32), gini=gini.astype(np.float32),
    ), xe, xo


_NC_CACHE = {}


def get_program():
    if "nc" not in _NC_CACHE:
        _NC_CACHE["nc"] = build_program()
    return _NC_CACHE["nc"]


def make_in_maps(inputs):
    shared, xe, xo = _prepare(inputs)
    return [dict(xe=np.ascontiguousarray(xe[c * B_LOC:(c + 1) * B_LOC]),
                 xo=np.ascontiguousarray(xo[c * B_LOC:(c + 1) * B_LOC]),
                 **shared)
            for c in range(N_CORES)]


def kernel(**inputs) -> np.ndarray:
    from concourse.bass_utils import run_bass_kernel_spmd

    nc = get_program()
    in_maps = make_in_maps(inputs)
    res = run_bass_kernel_spmd(nc, in_maps, list(range(N_CORES)))
    out = np.empty((B, L, H), dtype=np.float32)
    for c in range(N_CORES):
        sl = slice(c * B_LOC, (c + 1) * B_LOC)
        out[sl, 0::2, :] = res.results[c]["oute"].transpose(0, 2, 1)
        out[sl, 1::2, :] = res.results[c]["outo"].transpose(0, 2, 1)
    return out


# revision 9
# speedup vs baseline: 1.0541x; 1.0348x over previous
"""LRU (diagonal complex linear recurrence) Trainium2 Bass kernel, v10.

Math (per batch b, channel h, time t = 0..L-1):
    u_t   = delta * (x_t @ B_real + i * x_t @ B_img)
    h_t   = lam * h_{t-1} + u_t,   h_{-1} = h0,  lam = r e^{i theta}
    out_t = Re(h_t)

Structure — time-decimated (m=2) polar scan with the pair-combine
folded into the GEMM:
  E_j := h_{2j} obeys E_j = lam^2 E_{j-1} + u~_j with
  u~_j = lam*u_{2j-1} + u_{2j} = x_{2j-1} @ (lam*Bd) + x_{2j} @ Bd.
  So a GEMM over pairs (x_{2j-1}, x_{2j}) with weight sets
  {btr2,bti2} = lam*(btr+i*bti) and {btr,bti} yields u~ directly at
  half resolution. Polar trick on lam^2 = r^2 e^{i*2theta}:
  E_j = e^{i*2theta(j+1)} G_j,  G_j = r^2 G_{j-1} + e^{-i*2theta(j+1)} u~_j,
  G_{-1} = h0 / lam  (host-computed, fp32).
  Even outputs: out_{2j}  = C.Gr - S.Gi           (C,S = cos/sin 2theta(j+1))
  Odd  outputs: out_{2j+1} = P1.Gr - P2.Gi + Re(u_{2j+1})
                (P1,P2 = r*cos/sin(theta(2j+3)); Re(u_odd) from a third
                 partial GEMM x_{2j+1} @ btr).
  Scans run on DVE at half resolution (scan is the only serial resource).
  ALL elementwise ops run on DVE in fp16 2x mode; GpSimd left idle
  (SBUF port contention degrades DVE otherwise). PSUM->SBUF staging on
  ScalarE. Final even/odd combines run as identity matmuls on the PE.

v10 schedule (vs v9): weights packed pairwise and tables packed per-hg
into single DMAs; weights+tables dispatched on the Activation HWDGE
queue while x streams on the SP queue (parallel descriptor generation —
v9 serialized ~127 dispatches at ~0.65us each on SP, so the first DMA
byte moved at ~8.7us). Work units are interleaved (hg0,hg1) x (b0,b1)
x j-halves so the first GEMMs need only the first x quarter, and the
final stream is tapered (1024,512,512) to shorten the drain chain.

Sharding: batch-parallel over 8 cores (2 batch elements each), SPMD.
"""

from contextlib import ExitStack

import numpy as np

import concourse.bass as bass
import concourse.tile as tile
from concourse import bacc, mybir

B, L, F, H = 16, 4096, 512, 512
N_CORES = 8
B_LOC = B // N_CORES
HG = H // 128
FG = F // 128
J = L // 2            # half-res length
JOP = 2064            # odd stream padded (quarters of 528, 16-col overlap)
W = 1024              # elementwise tile width (j-cols)
PW = 512              # PSUM gemm tile width (j-cols)
FP32 = mybir.dt.float32
F16 = mybir.dt.float16
A = mybir.AluOpType


def build_program():
    nc = bacc.Bacc("TRN2", target_bir_lowering=False, debug=False,
                   enable_asserts=False, num_devices=1)

    xe_d = nc.dram_tensor("xe", [B_LOC, F, J], F16, kind="ExternalInput").ap()
    xo_d = nc.dram_tensor("xo", [B_LOC, F, JOP], F16, kind="ExternalInput").ap()
    # weight walls: [F, 2, H] = (btr2, btr) and (bti2, bti)
    wa_d = nc.dram_tensor("wall_a", [F, 2, H], F16, kind="ExternalInput").ap()
    wb_d = nc.dram_tensor("wall_b", [F, 2, H], F16, kind="ExternalInput").ap()
    # tables packed per hg: [HG, 128, 2, J] = (ctab, stab) and (p1tab, p2tab)
    cs_d = nc.dram_tensor("cs_tab", [HG, 128, 2, J], F16, kind="ExternalInput").ap()
    ps_d_t = nc.dram_tensor("ps_tab", [HG, 128, 2, J], F16, kind="ExternalInput").ap()
    r2_d = nc.dram_tensor("r2vec", [H], FP32, kind="ExternalInput").ap()
    gr0_d = nc.dram_tensor("ginr", [H], FP32, kind="ExternalInput").ap()
    gi0_d = nc.dram_tensor("gini", [H], FP32, kind="ExternalInput").ap()
    oute_d = nc.dram_tensor("oute", [B_LOC, H, J], F16, kind="ExternalOutput").ap()
    outo_d = nc.dram_tensor("outo", [B_LOC, H, J], F16, kind="ExternalOutput").ap()

    with tile.TileContext(nc) as tc, ExitStack() as ctx:
        singles = ctx.enter_context(tc.tile_pool(name="singles", bufs=1))
        xt_pool = ctx.enter_context(tc.tile_pool(name="xt", bufs=1))
        tab_pool = ctx.enter_context(tc.tile_pool(name="tabs", bufs=2))
        u_pool = ctx.enter_context(tc.tile_pool(name="u", bufs=2))
        work = ctx.enter_context(tc.tile_pool(name="work", bufs=2))
        opool = ctx.enter_context(tc.tile_pool(name="opool", bufs=2))
        gpool = ctx.enter_context(tc.tile_pool(name="gpool", bufs=3))
        rpool = ctx.enter_context(tc.tile_pool(name="res", bufs=3))
        ps_ab = ctx.enter_context(tc.tile_pool(name="ps_ab", bufs=1, space="PSUM"))
        ps_c = ctx.enter_context(tc.tile_pool(name="ps_c", bufs=4, space="PSUM"))
        ps_d = ctx.enter_context(tc.tile_pool(name="ps_d", bufs=2, space="PSUM"))

        # ---- weights: wall_a (pa/pc) heads the SP queue, wall_b (pb)
        # heads the scalar queue, so both land in parallel
        wall_a = singles.tile([128, FG, 2, H], F16)   # [., fg, 0,*]=btr2 [.,fg,1,*]=btr
        wall_b = singles.tile([128, FG, 2, H], F16)
        nc.sync.dma_start(out=wall_a,
                          in_=wa_d.rearrange("(fg p) t h -> p fg t h", p=128))
        nc.scalar.dma_start(out=wall_b,
                            in_=wb_d.rearrange("(fg p) t h -> p fg t h", p=128))

        # ---- x streams in half-J chunks (2KB dram rows for fast DMA):
        # xe halves on the SP queue, xo halves on the scalar queue
        xte = [[xt_pool.tile([128, FG, 1024], F16, tag=f"xte{b}_{h}",
                             name=f"xte{b}_{h}") for h in range(2)]
               for b in range(B_LOC)]
        xto = [[xt_pool.tile([128, FG, 1040], F16, tag=f"xto{b}_{h}",
                             name=f"xto{b}_{h}") for h in range(2)]
               for b in range(B_LOC)]

        def load_xe_half(b, h):
            xe_r = xe_d[b].rearrange("(fg p) j -> p fg j", p=128)
            nc.sync.dma_start(out=xte[b][h],
                              in_=xe_r[:, :, 1024 * h:1024 * h + 1024])

        def load_xo_half(b, h):
            xo_r = xo_d[b].rearrange("(fg p) j -> p fg j", p=128)
            nc.scalar.dma_start(out=xto[b][h],
                                in_=xo_r[:, :, 1024 * h:1024 * h + 1040])

        load_xe_half(0, 0)
        load_xe_half(0, 1)

        # ---- small vectors + identity (tiny; on sync between xe halves)
        r2_s = singles.tile([128, HG], FP32)
        gr0_s = singles.tile([128, HG], FP32)
        gi0_s = singles.tile([128, HG], FP32)
        nc.sync.dma_start(out=r2_s, in_=r2_d.rearrange("(hg p) -> p hg", p=128))
        nc.sync.dma_start(out=gr0_s, in_=gr0_d.rearrange("(hg p) -> p hg", p=128))
        nc.sync.dma_start(out=gi0_s, in_=gi0_d.rearrange("(hg p) -> p hg", p=128))
        from concourse.masks import make_identity
        ident = singles.tile([128, 128], F16)
        make_identity(nc, ident)
        nident = singles.tile([128, 128], F16)
        nc.vector.tensor_scalar(nident, ident, -1.0, None, op0=A.mult)

        r2bc = singles.tile([128, HG, W], FP32)
        nc.vector.memset(r2bc, 1.0)
        for hg in range(HG):
            nc.vector.tensor_scalar(r2bc[:, hg, :], r2bc[:, hg, :],
                                    r2_s[:, hg:hg + 1], None, op0=A.mult)

        # ---- tables: per-hg single dispatches on the scalar queue
        cs_t = {}
        ps_t = {}

        def alloc_tabs(hg):
            cs_t[hg] = tab_pool.tile([128, 2, J], F16, tag="cs",
                                     name=f"cs{hg}")
            ps_t[hg] = tab_pool.tile([128, 2, J], F16, tag="ps",
                                     name=f"ps{hg}")

        def load_tab_half(hg, half):
            jsl = slice(half * J // 2, (half + 1) * J // 2)
            nc.scalar.dma_start(out=cs_t[hg][:, :, jsl], in_=cs_d[hg][:, :, jsl])
            nc.scalar.dma_start(out=ps_t[hg][:, :, jsl], in_=ps_d_t[hg][:, :, jsl])

        # scalar queue order mirrors first-need times
        alloc_tabs(0)
        alloc_tabs(1)
        load_xo_half(0, 0)
        load_tab_half(0, 0)
        load_xo_half(0, 1)
        load_tab_half(1, 0)
        load_tab_half(0, 1)
        load_tab_half(1, 1)
        load_xo_half(1, 0)
        load_xo_half(1, 1)
        load_xe_half(1, 0)
        load_xe_half(1, 1)
        alloc_tabs(2)
        alloc_tabs(3)
        load_tab_half(2, 0)
        load_tab_half(3, 0)
        load_tab_half(2, 1)
        load_tab_half(3, 1)

        # ---- work unit list: (hg, b, j0, w). hg0/hg1 interleaved first so
        # early GEMMs need only early x quarters; final stream tapered.
        units = [
            (0, 0, 0, 1024), (1, 0, 0, 1024), (0, 0, 1024, 1024), (1, 0, 1024, 1024),
            (0, 1, 0, 1024), (1, 1, 0, 1024), (0, 1, 1024, 1024), (1, 1, 1024, 1024),
            (2, 0, 0, 1024), (3, 0, 0, 1024), (2, 0, 1024, 1024), (3, 0, 1024, 1024),
            (2, 1, 0, 1024), (3, 1, 0, 1024), (2, 1, 1024, 1024),
            (3, 1, 1024, 512), (3, 1, 1536, 512),
        ]

        pending = []

        def flush_one(p):
            for pc, ssl, sw in p["pcs"]:
                nc.tensor.matmul(pc[:, :sw], ident, p["o3"][:, ssl],
                                 start=False, stop=False)
                nc.tensor.matmul(pc[:, :sw], nident, p["o4"][:, ssl],
                                 start=False, stop=True)
                nc.scalar.copy(out=p["res_o"][:, ssl], in_=pc[:, :sw])
                pd = ps_d.tile([128, PW], FP32, tag="pd")
                nc.tensor.matmul(pd[:, :sw], ident, p["o1"][:, ssl],
                                 start=True, stop=False)
                nc.tensor.matmul(pd[:, :sw], nident, p["o2"][:, ssl],
                                 start=False, stop=True)
                nc.scalar.copy(out=p["res_e"][:, ssl], in_=pd[:, :sw])
            nc.sync.dma_start(out=oute_d[p["b"], p["hsl"], p["jsl"]],
                              in_=p["res_e"][:, :p["w"]])
            nc.sync.dma_start(out=outo_d[p["b"], p["hsl"], p["jsl"]],
                              in_=p["res_o"][:, :p["w"]])

        gprev = {}

        for ui, (hg, b, j0, w) in enumerate(units):
            hsl = slice(hg * 128, (hg + 1) * 128)
            if len(pending) >= 2:
                flush_one(pending.pop(0))
            ur_sb = u_pool.tile([128, W], F16, tag="ur_sb")
            ui_sb = u_pool.tile([128, W], F16, tag="ui_sb")
            pcs = []
            nps = (w + PW - 1) // PW
            for ps in range(nps):
                p0 = j0 + ps * PW
                sw = min(PW, w - ps * PW)
                xh = p0 // 1024               # x half tile
                lo = p0 - 1024 * xh           # offset within half
                osl = slice(lo, lo + sw)
                osl1 = slice(lo + 1, lo + sw + 1)
                ssl = slice(ps * PW, ps * PW + sw)
                pa = ps_ab.tile([128, PW], FP32, tag="pa")
                pb = ps_ab.tile([128, PW], FP32, tag="pb")
                pc = ps_c.tile([128, PW], FP32, tag="pc")
                for fg in range(FG):
                    nc.tensor.matmul(pa[:, :sw], wall_a[:, fg, 0, hsl],
                                     xto[b][xh][:, fg, osl],
                                     start=(fg == 0), stop=False)
                for fg in range(FG):
                    nc.tensor.matmul(pa[:, :sw], wall_a[:, fg, 1, hsl],
                                     xte[b][xh][:, fg, osl],
                                     start=False, stop=(fg == FG - 1))
                for fg in range(FG):
                    nc.tensor.matmul(pb[:, :sw], wall_b[:, fg, 0, hsl],
                                     xto[b][xh][:, fg, osl],
                                     start=(fg == 0), stop=False)
                for fg in range(FG):
                    nc.tensor.matmul(pb[:, :sw], wall_b[:, fg, 1, hsl],
                                     xte[b][xh][:, fg, osl],
                                     start=False, stop=(fg == FG - 1))
                for fg in range(FG):
                    nc.tensor.matmul(pc[:, :sw], wall_a[:, fg, 1, hsl],
                                     xto[b][xh][:, fg, osl1],
                                     start=(fg == 0), stop=False)
                nc.scalar.copy(out=ur_sb[:, ssl], in_=pa[:, :sw])
                nc.scalar.copy(out=ui_sb[:, ssl], in_=pb[:, :sw])
                pcs.append((pc, ssl, sw))

            jsl = slice(j0, j0 + w)
            cw = cs_t[hg][:, 0, jsl]
            sw = cs_t[hg][:, 1, jsl]
            # input rotation: v = e^{-i*2theta(j+1)} u~
            t1 = work.tile([128, W], F16, tag="t1")
            t2 = work.tile([128, W], F16, tag="t2")
            t3 = work.tile([128, W], F16, tag="t3")
            t4 = work.tile([128, W], F16, tag="t4")
            vr = work.tile([128, W], F16, tag="vr")
            vi = work.tile([128, W], F16, tag="vi")
            nc.vector.tensor_mul(t1[:, :w], cw, ur_sb[:, :w])
            nc.vector.tensor_mul(t2[:, :w], sw, ui_sb[:, :w])
            nc.vector.tensor_add(vr[:, :w], t1[:, :w], t2[:, :w])
            nc.vector.tensor_mul(t3[:, :w], cw, ui_sb[:, :w])
            nc.vector.tensor_mul(t4[:, :w], sw, ur_sb[:, :w])
            nc.vector.tensor_sub(vi[:, :w], t3[:, :w], t4[:, :w])

            gr = gpool.tile([128, W], F16, tag="gr")
            gi = gpool.tile([128, W], F16, tag="gi")
            if j0 == 0:
                init_r = gr0_s[:, hg:hg + 1]
                init_i = gi0_s[:, hg:hg + 1]
            else:
                gr_p, gi_p, wp = gprev[(hg, b)]
                init_r = gr_p[:, wp - 1:wp]
                init_i = gi_p[:, wp - 1:wp]
            nc.vector.tensor_tensor_scan(gr[:, :w], r2bc[:, hg, :w],
                                         vr[:, :w], init_r,
                                         op0=A.mult, op1=A.add)
            nc.vector.tensor_tensor_scan(gi[:, :w], r2bc[:, hg, :w],
                                         vi[:, :w], init_i,
                                         op0=A.mult, op1=A.add)
            gprev[(hg, b)] = (gr, gi, w)

            # output: even t=2j -> C.Gr - S.Gi ; odd -> P1.Gr - P2.Gi + uo
            o1 = opool.tile([128, W], F16, tag="o1")
            o2 = opool.tile([128, W], F16, tag="o2")
            o3 = opool.tile([128, W], F16, tag="o3")
            o4 = opool.tile([128, W], F16, tag="o4")
            res_e = rpool.tile([128, W], F16, tag="res_e")
            res_o = rpool.tile([128, W], F16, tag="res_o")
            nc.vector.tensor_mul(o1[:, :w], cw, gr[:, :w])
            nc.vector.tensor_mul(o2[:, :w], sw, gi[:, :w])
            nc.vector.tensor_mul(o3[:, :w], ps_t[hg][:, 0, jsl], gr[:, :w])
            nc.vector.tensor_mul(o4[:, :w], ps_t[hg][:, 1, jsl], gi[:, :w])
            pending.append(dict(pcs=pcs, o1=o1, o2=o2, o3=o3, o4=o4,
                                res_e=res_e, res_o=res_o,
                                b=b, hsl=hsl, jsl=jsl, w=w))

        while pending:
            flush_one(pending.pop(0))

    nc.compile()
    return nc


def _prepare(inputs):
    x = np.asarray(inputs["x"], dtype=np.float32)
    B_real = np.asarray(inputs["B_real"], dtype=np.float64)
    B_img = np.asarray(inputs["B_img"], dtype=np.float64)
    nu = np.asarray(inputs["nu"], dtype=np.float64)
    theta = np.asarray(inputs["theta"], dtype=np.float64)
    delta = np.asarray(inputs["delta"], dtype=np.float64)
    h0r = np.asarray(inputs["h0_real"], dtype=np.float64)
    h0i = np.asarray(inputs["h0_img"], dtype=np.float64)

    r = np.exp(-np.exp(nu))
    btr = B_real * delta[None, :]
    bti = B_img * delta[None, :]
    rc = r * np.cos(theta)
    rs = r * np.sin(theta)
    btr2 = btr * rc[None, :] - bti * rs[None, :]
    bti2 = btr * rs[None, :] + bti * rc[None, :]

    jj = np.arange(J, dtype=np.float64)
    ang_c = theta[:, None] * (2.0 * jj + 2.0)[None, :]   # 2theta(j+1)
    ctab = np.cos(ang_c)
    stab = np.sin(ang_c)
    ang_p = theta[:, None] * (2.0 * jj + 3.0)[None, :]   # theta(2j+3)
    p1 = r[:, None] * np.cos(ang_p)
    p2 = r[:, None] * np.sin(ang_p)

    # G_{-1} = h0 / lam = h0 * e^{-i theta} / r
    ginr = (h0r * np.cos(theta) + h0i * np.sin(theta)) / r
    gini = (h0i * np.cos(theta) - h0r * np.sin(theta)) / r

    xh = x.astype(np.float16)
    # host-side transpose to [B, F, J] so device loads are contiguous
    xe = np.ascontiguousarray(xh[:, 0::2, :].transpose(0, 2, 1))   # (B, F, J)
    xo = np.zeros((B, F, JOP), dtype=np.float16)
    xo[:, :, 1:J + 1] = xh[:, 1::2, :].transpose(0, 2, 1)          # slot p = x_{2p-1}

    wall_a = np.stack([btr2, btr], axis=1).astype(np.float16)      # (F, 2, H)
    wall_b = np.stack([bti2, bti], axis=1).astype(np.float16)
    cth = ctab.reshape(HG, 128, J)
    sth = stab.reshape(HG, 128, J)
    cs_tab = np.stack([cth, sth], axis=2).astype(np.float16)       # (HG,128,2,J)
    p1h = p1.reshape(HG, 128, J)
    p2h = p2.reshape(HG, 128, J)
    ps_tab = np.stack([p1h, p2h], axis=2).astype(np.float16)

    return dict(
        wall_a=wall_a, wall_b=wall_b,
        r2vec=(r * r).astype(np.float32),
        cs_tab=cs_tab, ps_tab=ps_tab,
        ginr=ginr.astype(np.float32), gini=gini.astype(np.float32),
    ), xe, xo


_NC_CACHE = {}


def get_program():
    if "nc" not in _NC_CACHE:
        _NC_CACHE["nc"] = build_program()
    return _NC_CACHE["nc"]


def make_in_maps(inputs):
    shared, xe, xo = _prepare(inputs)
    return [dict(xe=np.ascontiguousarray(xe[c * B_LOC:(c + 1) * B_LOC]),
                 xo=np.ascontiguousarray(xo[c * B_LOC:(c + 1) * B_LOC]),
                 **shared)
            for c in range(N_CORES)]


def kernel(**inputs) -> np.ndarray:
    from concourse.bass_utils import run_bass_kernel_spmd

    nc = get_program()
    in_maps = make_in_maps(inputs)
    res = run_bass_kernel_spmd(nc, in_maps, list(range(N_CORES)))
    out = np.empty((B, L, H), dtype=np.float32)
    for c in range(N_CORES):
        sl = slice(c * B_LOC, (c + 1) * B_LOC)
        out[sl, 0::2, :] = res.results[c]["oute"].transpose(0, 2, 1)
        out[sl, 1::2, :] = res.results[c]["outo"].transpose(0, 2, 1)
    return out


# revision 10
# speedup vs baseline: 1.0608x; 1.0064x over previous
"""LRU (diagonal complex linear recurrence) Trainium2 Bass kernel, v10.

Math (per batch b, channel h, time t = 0..L-1):
    u_t   = delta * (x_t @ B_real + i * x_t @ B_img)
    h_t   = lam * h_{t-1} + u_t,   h_{-1} = h0,  lam = r e^{i theta}
    out_t = Re(h_t)

Structure — time-decimated (m=2) polar scan with the pair-combine
folded into the GEMM:
  E_j := h_{2j} obeys E_j = lam^2 E_{j-1} + u~_j with
  u~_j = lam*u_{2j-1} + u_{2j} = x_{2j-1} @ (lam*Bd) + x_{2j} @ Bd.
  So a GEMM over pairs (x_{2j-1}, x_{2j}) with weight sets
  {btr2,bti2} = lam*(btr+i*bti) and {btr,bti} yields u~ directly at
  half resolution. Polar trick on lam^2 = r^2 e^{i*2theta}:
  E_j = e^{i*2theta(j+1)} G_j,  G_j = r^2 G_{j-1} + e^{-i*2theta(j+1)} u~_j,
  G_{-1} = h0 / lam  (host-computed, fp32).
  Even outputs: out_{2j}  = C.Gr - S.Gi           (C,S = cos/sin 2theta(j+1))
  Odd  outputs: out_{2j+1} = P1.Gr - P2.Gi + Re(u_{2j+1})
                (P1,P2 = r*cos/sin(theta(2j+3)); Re(u_odd) from a third
                 partial GEMM x_{2j+1} @ btr).
  Scans run on DVE at half resolution (scan is the only serial resource).
  ALL elementwise ops run on DVE in fp16 2x mode; GpSimd left idle
  (SBUF port contention degrades DVE otherwise). PSUM->SBUF staging on
  ScalarE. Final even/odd combines run as identity matmuls on the PE.

v10 schedule (vs v9): weights packed pairwise and tables packed per-hg
into single DMAs; weights+tables dispatched on the Activation HWDGE
queue while x streams on the SP queue (parallel descriptor generation —
v9 serialized ~127 dispatches at ~0.65us each on SP, so the first DMA
byte moved at ~8.7us). Work units are interleaved (hg0,hg1) x (b0,b1)
x j-halves so the first GEMMs need only the first x quarter, and the
final stream is tapered (1024,512,512) to shorten the drain chain.

Sharding: batch-parallel over 8 cores (2 batch elements each), SPMD.
"""

from contextlib import ExitStack

import numpy as np

import concourse.bass as bass
import concourse.tile as tile
from concourse import bacc, mybir

B, L, F, H = 16, 4096, 512, 512
N_CORES = 8
B_LOC = B // N_CORES
HG = H // 128
FG = F // 128
J = L // 2            # half-res length
JOP = 2064            # odd stream padded (quarters of 528, 16-col overlap)
W = 1024              # elementwise tile width (j-cols)
PW = 512              # PSUM gemm tile width (j-cols)
FP32 = mybir.dt.float32
F16 = mybir.dt.float16
A = mybir.AluOpType


def build_program():
    nc = bacc.Bacc("TRN2", target_bir_lowering=False, debug=False,
                   enable_asserts=False, num_devices=1)

    xe_d = nc.dram_tensor("xe", [B_LOC, F, J], F16, kind="ExternalInput").ap()
    xo_d = nc.dram_tensor("xo", [B_LOC, F, JOP], F16, kind="ExternalInput").ap()
    # weight walls: [F, 2, H] = (btr2, btr) and (bti2, bti)
    wa_d = nc.dram_tensor("wall_a", [F, 2, H], F16, kind="ExternalInput").ap()
    wb_d = nc.dram_tensor("wall_b", [F, 2, H], F16, kind="ExternalInput").ap()
    # tables packed per hg: [HG, 128, 2, J] = (ctab, stab) and (p1tab, p2tab)
    cs_d = nc.dram_tensor("cs_tab", [HG, 128, 2, J], F16, kind="ExternalInput").ap()
    ps_d_t = nc.dram_tensor("ps_tab", [HG, 128, 2, J], F16, kind="ExternalInput").ap()
    r2_d = nc.dram_tensor("r2vec", [H], FP32, kind="ExternalInput").ap()
    gr0_d = nc.dram_tensor("ginr", [H], FP32, kind="ExternalInput").ap()
    gi0_d = nc.dram_tensor("gini", [H], FP32, kind="ExternalInput").ap()
    oute_d = nc.dram_tensor("oute", [B_LOC, H, J], F16, kind="ExternalOutput").ap()
    outo_d = nc.dram_tensor("outo", [B_LOC, H, J], F16, kind="ExternalOutput").ap()

    with tile.TileContext(nc) as tc, ExitStack() as ctx:
        singles = ctx.enter_context(tc.tile_pool(name="singles", bufs=1))
        xt_pool = ctx.enter_context(tc.tile_pool(name="xt", bufs=1))
        tab_pool = ctx.enter_context(tc.tile_pool(name="tabs", bufs=2))
        u_pool = ctx.enter_context(tc.tile_pool(name="u", bufs=2))
        work = ctx.enter_context(tc.tile_pool(name="work", bufs=2))
        opool = ctx.enter_context(tc.tile_pool(name="opool", bufs=2))
        gpool = ctx.enter_context(tc.tile_pool(name="gpool", bufs=3))
        rpool = ctx.enter_context(tc.tile_pool(name="res", bufs=3))
        ps_ab = ctx.enter_context(tc.tile_pool(name="ps_ab", bufs=1, space="PSUM"))
        ps_c = ctx.enter_context(tc.tile_pool(name="ps_c", bufs=4, space="PSUM"))
        ps_d = ctx.enter_context(tc.tile_pool(name="ps_d", bufs=2, space="PSUM"))

        # ---- weights: wall_a (pa/pc) on the SP queue, wall_b (pb) on the
        # scalar queue, per-fg chunks interleaved with the first x chunks so
        # the first matmuls (subtile deps) start as early as possible
        wall_a = singles.tile([128, FG, 2, H], F16)   # [., fg, 0,*]=btr2 [.,fg,1,*]=btr
        wall_b = singles.tile([128, FG, 2, H], F16)
        wa_r = wa_d.rearrange("(fg p) t h -> p fg t h", p=128)
        wb_r = wb_d.rearrange("(fg p) t h -> p fg t h", p=128)

        # ---- x streams in half-J chunks (2KB dram rows for fast DMA):
        # xe halves on the SP queue, xo halves on the scalar queue
        xte = [[xt_pool.tile([128, FG, 1024], F16, tag=f"xte{b}_{h}",
                             name=f"xte{b}_{h}") for h in range(2)]
               for b in range(B_LOC)]
        xto = [[xt_pool.tile([128, FG, 1040], F16, tag=f"xto{b}_{h}",
                             name=f"xto{b}_{h}") for h in range(2)]
               for b in range(B_LOC)]

        def load_xe_half(b, h, fg=None):
            xe_r = xe_d[b].rearrange("(fg p) j -> p fg j", p=128)
            jsl = slice(1024 * h, 1024 * h + 1024)
            if fg is None:
                nc.sync.dma_start(out=xte[b][h][:, :, :], in_=xe_r[:, :, jsl])
            else:
                nc.sync.dma_start(out=xte[b][h][:, fg, :],
                                  in_=xe_r[:, fg, jsl])

        def load_xo_half(b, h, fg=None):
            xo_r = xo_d[b].rearrange("(fg p) j -> p fg j", p=128)
            jsl = slice(1024 * h, 1024 * h + 1040)
            if fg is None:
                nc.scalar.dma_start(out=xto[b][h][:, :, :], in_=xo_r[:, :, jsl])
            else:
                nc.scalar.dma_start(out=xto[b][h][:, fg, :],
                                    in_=xo_r[:, fg, jsl])

        # interleave weight-fg and x-fg chunks on both queues
        for fg in range(FG):
            nc.sync.dma_start(out=wall_a[:, fg], in_=wa_r[:, fg])
            load_xe_half(0, 0, fg)
        load_xe_half(0, 1)

        # ---- small vectors + identity (tiny; on sync between xe halves)
        r2_s = singles.tile([128, HG], FP32)
        gr0_s = singles.tile([128, HG], FP32)
        gi0_s = singles.tile([128, HG], FP32)
        nc.sync.dma_start(out=r2_s, in_=r2_d.rearrange("(hg p) -> p hg", p=128))
        nc.sync.dma_start(out=gr0_s, in_=gr0_d.rearrange("(hg p) -> p hg", p=128))
        nc.sync.dma_start(out=gi0_s, in_=gi0_d.rearrange("(hg p) -> p hg", p=128))
        from concourse.masks import make_identity
        ident = singles.tile([128, 128], F16)
        make_identity(nc, ident)
        nident = singles.tile([128, 128], F16)
        nc.vector.tensor_scalar(nident, ident, -1.0, None, op0=A.mult)

        r2bc = singles.tile([128, HG, W], FP32)
        nc.vector.memset(r2bc, 1.0)
        for hg in range(HG):
            nc.vector.tensor_scalar(r2bc[:, hg, :], r2bc[:, hg, :],
                                    r2_s[:, hg:hg + 1], None, op0=A.mult)

        # ---- tables: per-hg single dispatches on the scalar queue
        cs_t = {}
        ps_t = {}

        def alloc_tabs(hg):
            cs_t[hg] = tab_pool.tile([128, 2, J], F16, tag="cs",
                                     name=f"cs{hg}")
            ps_t[hg] = tab_pool.tile([128, 2, J], F16, tag="ps",
                                     name=f"ps{hg}")

        def load_cs_half(hg, half):
            jsl = slice(half * J // 2, (half + 1) * J // 2)
            nc.scalar.dma_start(out=cs_t[hg][:, :, jsl], in_=cs_d[hg][:, :, jsl])

        def load_ps_half(hg, half):
            jsl = slice(half * J // 2, (half + 1) * J // 2)
            nc.scalar.dma_start(out=ps_t[hg][:, :, jsl], in_=ps_d_t[hg][:, :, jsl])

        # scalar queue order mirrors first-need times
        alloc_tabs(0)
        alloc_tabs(1)
        for fg in range(FG):
            nc.scalar.dma_start(out=wall_b[:, fg], in_=wb_r[:, fg])
            load_xo_half(0, 0, fg)
        load_cs_half(0, 0)
        load_xo_half(0, 1)
        load_ps_half(0, 0)
        load_cs_half(1, 0)
        load_ps_half(1, 0)
        load_xo_half(1, 0)
        load_cs_half(0, 1)
        load_ps_half(0, 1)
        load_cs_half(1, 1)
        load_ps_half(1, 1)
        load_xo_half(1, 1)
        load_xe_half(1, 0)
        load_xe_half(1, 1)
        alloc_tabs(2)
        alloc_tabs(3)
        load_cs_half(2, 0)
        load_ps_half(2, 0)
        load_cs_half(3, 0)
        load_ps_half(3, 0)
        load_cs_half(2, 1)
        load_ps_half(2, 1)
        load_cs_half(3, 1)
        load_ps_half(3, 1)

        # ---- work unit list: (hg, b, j0, w). hg0/hg1 interleaved first so
        # early GEMMs need only early x quarters; final stream tapered.
        units = [
            (0, 0, 0, 1024), (1, 0, 0, 1024), (0, 0, 1024, 1024), (1, 0, 1024, 1024),
            (0, 1, 0, 1024), (1, 1, 0, 1024), (0, 1, 1024, 1024), (1, 1, 1024, 1024),
            (2, 0, 0, 1024), (3, 0, 0, 1024), (2, 0, 1024, 1024), (3, 0, 1024, 1024),
            (2, 1, 0, 1024), (3, 1, 0, 1024), (2, 1, 1024, 1024),
            (3, 1, 1024, 512), (3, 1, 1536, 512),
        ]

        pending = []

        def flush_one(p):
            for pc, ssl, sw in p["pcs"]:
                nc.tensor.matmul(pc[:, :sw], ident, p["o3"][:, ssl],
                                 start=False, stop=False)
                nc.tensor.matmul(pc[:, :sw], nident, p["o4"][:, ssl],
                                 start=False, stop=True)
                nc.scalar.copy(out=p["res_o"][:, ssl], in_=pc[:, :sw])
                pd = ps_d.tile([128, PW], FP32, tag="pd")
                nc.tensor.matmul(pd[:, :sw], ident, p["o1"][:, ssl],
                                 start=True, stop=False)
                nc.tensor.matmul(pd[:, :sw], nident, p["o2"][:, ssl],
                                 start=False, stop=True)
                nc.scalar.copy(out=p["res_e"][:, ssl], in_=pd[:, :sw])
            nc.sync.dma_start(out=oute_d[p["b"], p["hsl"], p["jsl"]],
                              in_=p["res_e"][:, :p["w"]])
            nc.scalar.dma_start(out=outo_d[p["b"], p["hsl"], p["jsl"]],
                                in_=p["res_o"][:, :p["w"]])

        gprev = {}

        for ui, (hg, b, j0, w) in enumerate(units):
            hsl = slice(hg * 128, (hg + 1) * 128)
            if len(pending) >= 2:
                flush_one(pending.pop(0))
            ur_sb = u_pool.tile([128, W], F16, tag="ur_sb")
            ui_sb = u_pool.tile([128, W], F16, tag="ui_sb")
            pcs = []
            nps = (w + PW - 1) // PW
            for ps in range(nps):
                p0 = j0 + ps * PW
                sw = min(PW, w - ps * PW)
                xh = p0 // 1024               # x half tile
                lo = p0 - 1024 * xh           # offset within half
                osl = slice(lo, lo + sw)
                osl1 = slice(lo + 1, lo + sw + 1)
                ssl = slice(ps * PW, ps * PW + sw)
                pa = ps_ab.tile([128, PW], FP32, tag="pa")
                pb = ps_ab.tile([128, PW], FP32, tag="pb")
                pc = ps_c.tile([128, PW], FP32, tag="pc")
                for fg in range(FG):
                    nc.tensor.matmul(pa[:, :sw], wall_a[:, fg, 0, hsl],
                                     xto[b][xh][:, fg, osl],
                                     start=(fg == 0), stop=False)
                for fg in range(FG):
                    nc.tensor.matmul(pa[:, :sw], wall_a[:, fg, 1, hsl],
                                     xte[b][xh][:, fg, osl],
                                     start=False, stop=(fg == FG - 1))
                for fg in range(FG):
                    nc.tensor.matmul(pb[:, :sw], wall_b[:, fg, 0, hsl],
                                     xto[b][xh][:, fg, osl],
                                     start=(fg == 0), stop=False)
                for fg in range(FG):
                    nc.tensor.matmul(pb[:, :sw], wall_b[:, fg, 1, hsl],
                                     xte[b][xh][:, fg, osl],
                                     start=False, stop=(fg == FG - 1))
                for fg in range(FG):
                    nc.tensor.matmul(pc[:, :sw], wall_a[:, fg, 1, hsl],
                                     xto[b][xh][:, fg, osl1],
                                     start=(fg == 0), stop=False)
                nc.scalar.copy(out=ur_sb[:, ssl], in_=pa[:, :sw])
                nc.scalar.copy(out=ui_sb[:, ssl], in_=pb[:, :sw])
                pcs.append((pc, ssl, sw))

            jsl = slice(j0, j0 + w)
            cw = cs_t[hg][:, 0, jsl]
            sw = cs_t[hg][:, 1, jsl]
            # input rotation: v = e^{-i*2theta(j+1)} u~
            t1 = work.tile([128, W], F16, tag="t1")
            t2 = work.tile([128, W], F16, tag="t2")
            t3 = work.tile([128, W], F16, tag="t3")
            t4 = work.tile([128, W], F16, tag="t4")
            vr = work.tile([128, W], F16, tag="vr")
            vi = work.tile([128, W], F16, tag="vi")
            nc.vector.tensor_mul(t1[:, :w], cw, ur_sb[:, :w])
            nc.vector.tensor_mul(t2[:, :w], sw, ui_sb[:, :w])
            nc.vector.tensor_add(vr[:, :w], t1[:, :w], t2[:, :w])
            nc.vector.tensor_mul(t3[:, :w], cw, ui_sb[:, :w])
            nc.vector.tensor_mul(t4[:, :w], sw, ur_sb[:, :w])
            nc.vector.tensor_sub(vi[:, :w], t3[:, :w], t4[:, :w])

            gr = gpool.tile([128, W], F16, tag="gr")
            gi = gpool.tile([128, W], F16, tag="gi")
            if j0 == 0:
                init_r = gr0_s[:, hg:hg + 1]
                init_i = gi0_s[:, hg:hg + 1]
            else:
                gr_p, gi_p, wp = gprev[(hg, b)]
                init_r = gr_p[:, wp - 1:wp]
                init_i = gi_p[:, wp - 1:wp]
            nc.vector.tensor_tensor_scan(gr[:, :w], r2bc[:, hg, :w],
                                         vr[:, :w], init_r,
                                         op0=A.mult, op1=A.add)
            nc.vector.tensor_tensor_scan(gi[:, :w], r2bc[:, hg, :w],
                                         vi[:, :w], init_i,
                                         op0=A.mult, op1=A.add)
            gprev[(hg, b)] = (gr, gi, w)

            # output: even t=2j -> C.Gr - S.Gi ; odd -> P1.Gr - P2.Gi + uo
            o1 = opool.tile([128, W], F16, tag="o1")
            o2 = opool.tile([128, W], F16, tag="o2")
            o3 = opool.tile([128, W], F16, tag="o3")
            o4 = opool.tile([128, W], F16, tag="o4")
            res_e = rpool.tile([128, W], F16, tag="res_e")
            res_o = rpool.tile([128, W], F16, tag="res_o")
            nc.vector.tensor_mul(o1[:, :w], cw, gr[:, :w])
            nc.vector.tensor_mul(o2[:, :w], sw, gi[:, :w])
            nc.vector.tensor_mul(o3[:, :w], ps_t[hg][:, 0, jsl], gr[:, :w])
            nc.vector.tensor_mul(o4[:, :w], ps_t[hg][:, 1, jsl], gi[:, :w])
            pending.append(dict(pcs=pcs, o1=o1, o2=o2, o3=o3, o4=o4,
                                res_e=res_e, res_o=res_o,
                                b=b, hsl=hsl, jsl=jsl, w=w))

        while pending:
            flush_one(pending.pop(0))

    nc.compile()
    return nc


def _prepare(inputs):
    x = np.asarray(inputs["x"], dtype=np.float32)
    B_real = np.asarray(inputs["B_real"], dtype=np.float64)
    B_img = np.asarray(inputs["B_img"], dtype=np.float64)
    nu = np.asarray(inputs["nu"], dtype=np.float64)
    theta = np.asarray(inputs["theta"], dtype=np.float64)
    delta = np.asarray(inputs["delta"], dtype=np.float64)
    h0r = np.asarray(inputs["h0_real"], dtype=np.float64)
    h0i = np.asarray(inputs["h0_img"], dtype=np.float64)

    r = np.exp(-np.exp(nu))
    btr = B_real * delta[None, :]
    bti = B_img * delta[None, :]
    rc = r * np.cos(theta)
    rs = r * np.sin(theta)
    btr2 = btr * rc[None, :] - bti * rs[None, :]
    bti2 = btr * rs[None, :] + bti * rc[None, :]

    jj = np.arange(J, dtype=np.float64)
    ang_c = theta[:, None] * (2.0 * jj + 2.0)[None, :]   # 2theta(j+1)
    ctab = np.cos(ang_c)
    stab = np.sin(ang_c)
    ang_p = theta[:, None] * (2.0 * jj + 3.0)[None, :]   # theta(2j+3)
    p1 = r[:, None] * np.cos(ang_p)
    p2 = r[:, None] * np.sin(ang_p)

    # G_{-1} = h0 / lam = h0 * e^{-i theta} / r
    ginr = (h0r * np.cos(theta) + h0i * np.sin(theta)) / r
    gini = (h0i * np.cos(theta) - h0r * np.sin(theta)) / r

    xh = x.astype(np.float16)
    # host-side transpose to [B, F, J] so device loads are contiguous
    xe = np.ascontiguousarray(xh[:, 0::2, :].transpose(0, 2, 1))   # (B, F, J)
    xo = np.zeros((B, F, JOP), dtype=np.float16)
    xo[:, :, 1:J + 1] = xh[:, 1::2, :].transpose(0, 2, 1)          # slot p = x_{2p-1}

    wall_a = np.stack([btr2, btr], axis=1).astype(np.float16)      # (F, 2, H)
    wall_b = np.stack([bti2, bti], axis=1).astype(np.float16)
    cth = ctab.reshape(HG, 128, J)
    sth = stab.reshape(HG, 128, J)
    cs_tab = np.stack([cth, sth], axis=2).astype(np.float16)       # (HG,128,2,J)
    p1h = p1.reshape(HG, 128, J)
    p2h = p2.reshape(HG, 128, J)
    ps_tab = np.stack([p1h, p2h], axis=2).astype(np.float16)

    return dict(
        wall_a=wall_a, wall_b=wall_b,
        r2vec=(r * r).astype(np.float32),
        cs_tab=cs_tab, ps_tab=ps_tab,
        ginr=ginr.astype(np.float32), gini=gini.astype(np.float32),
    ), xe, xo


_NC_CACHE = {}


def get_program():
    if "nc" not in _NC_CACHE:
        _NC_CACHE["nc"] = build_program()
    return _NC_CACHE["nc"]


def make_in_maps(inputs):
    shared, xe, xo = _prepare(inputs)
    return [dict(xe=np.ascontiguousarray(xe[c * B_LOC:(c + 1) * B_LOC]),
                 xo=np.ascontiguousarray(xo[c * B_LOC:(c + 1) * B_LOC]),
                 **shared)
            for c in range(N_CORES)]


def kernel(**inputs) -> np.ndarray:
    from concourse.bass_utils import run_bass_kernel_spmd

    nc = get_program()
    in_maps = make_in_maps(inputs)
    res = run_bass_kernel_spmd(nc, in_maps, list(range(N_CORES)))
    out = np.empty((B, L, H), dtype=np.float32)
    for c in range(N_CORES):
        sl = slice(c * B_LOC, (c + 1) * B_LOC)
        out[sl, 0::2, :] = res.results[c]["oute"].transpose(0, 2, 1)
        out[sl, 1::2, :] = res.results[c]["outo"].transpose(0, 2, 1)
    return out


# revision 11
# speedup vs baseline: 1.1228x; 1.0584x over previous
"""LRU (diagonal complex linear recurrence) Trainium2 Bass kernel, v10.

Math (per batch b, channel h, time t = 0..L-1):
    u_t   = delta * (x_t @ B_real + i * x_t @ B_img)
    h_t   = lam * h_{t-1} + u_t,   h_{-1} = h0,  lam = r e^{i theta}
    out_t = Re(h_t)

Structure — time-decimated (m=2) polar scan with the pair-combine
folded into the GEMM:
  E_j := h_{2j} obeys E_j = lam^2 E_{j-1} + u~_j with
  u~_j = lam*u_{2j-1} + u_{2j} = x_{2j-1} @ (lam*Bd) + x_{2j} @ Bd.
  So a GEMM over pairs (x_{2j-1}, x_{2j}) with weight sets
  {btr2,bti2} = lam*(btr+i*bti) and {btr,bti} yields u~ directly at
  half resolution. Polar trick on lam^2 = r^2 e^{i*2theta}:
  E_j = e^{i*2theta(j+1)} G_j,  G_j = r^2 G_{j-1} + e^{-i*2theta(j+1)} u~_j,
  G_{-1} = h0 / lam  (host-computed, fp32).
  Even outputs: out_{2j}  = C.Gr - S.Gi           (C,S = cos/sin 2theta(j+1))
  Odd  outputs: out_{2j+1} = P1.Gr - P2.Gi + Re(u_{2j+1})
                (P1,P2 = r*cos/sin(theta(2j+3)); Re(u_odd) from a third
                 partial GEMM x_{2j+1} @ btr).
  Scans run on DVE at half resolution (scan is the only serial resource).
  ALL elementwise ops run on DVE in fp16 2x mode; GpSimd left idle
  (SBUF port contention degrades DVE otherwise). PSUM->SBUF staging on
  ScalarE. Final even/odd combines run as identity matmuls on the PE.

v10 schedule (vs v9): weights packed pairwise and tables packed per-hg
into single DMAs; weights+tables dispatched on the Activation HWDGE
queue while x streams on the SP queue (parallel descriptor generation —
v9 serialized ~127 dispatches at ~0.65us each on SP, so the first DMA
byte moved at ~8.7us). Work units are interleaved (hg0,hg1) x (b0,b1)
x j-halves so the first GEMMs need only the first x quarter, and the
final stream is tapered (1024,512,512) to shorten the drain chain.

Sharding: batch-parallel over 8 cores (2 batch elements each), SPMD.
"""

from contextlib import ExitStack

import numpy as np

import concourse.bass as bass
import concourse.tile as tile
from concourse import bacc, mybir

B, L, F, H = 16, 4096, 512, 512
N_CORES = 8
B_LOC = B // N_CORES
HG = H // 128
FG = F // 128
J = L // 2            # half-res length
JOP = 2064            # odd stream padded (quarters of 528, 16-col overlap)
W = 1024              # elementwise tile width (j-cols)
PW = 512              # PSUM gemm tile width (j-cols)
FP32 = mybir.dt.float32
F16 = mybir.dt.float16
A = mybir.AluOpType


def build_program():
    nc = bacc.Bacc("TRN2", target_bir_lowering=False, debug=False,
                   enable_asserts=False, num_devices=1)

    xe_d = nc.dram_tensor("xe", [B_LOC, F, J], F16, kind="ExternalInput").ap()
    xo_d = nc.dram_tensor("xo", [B_LOC, F, JOP], F16, kind="ExternalInput").ap()
    # weight walls: [F, 2, H] = (btr2, btr) and (bti2, bti)
    wa_d = nc.dram_tensor("wall_a", [F, 2, H], F16, kind="ExternalInput").ap()
    wb_d = nc.dram_tensor("wall_b", [F, 2, H], F16, kind="ExternalInput").ap()
    # tables packed per hg: [HG, 128, 2, J] = (ctab, stab) and (p1tab, p2tab)
    cs_d = nc.dram_tensor("cs_tab", [HG, 128, 2, J], F16, kind="ExternalInput").ap()
    ps_d_t = nc.dram_tensor("ps_tab", [HG, 128, 2, J], F16, kind="ExternalInput").ap()
    r2_d = nc.dram_tensor("r2vec", [H], FP32, kind="ExternalInput").ap()
    gr0_d = nc.dram_tensor("ginr", [H], FP32, kind="ExternalInput").ap()
    gi0_d = nc.dram_tensor("gini", [H], FP32, kind="ExternalInput").ap()
    oute_d = nc.dram_tensor("oute", [B_LOC, H, J], F16, kind="ExternalOutput").ap()
    outo_d = nc.dram_tensor("outo", [B_LOC, H, J], F16, kind="ExternalOutput").ap()

    with tile.TileContext(nc) as tc, ExitStack() as ctx:
        singles = ctx.enter_context(tc.tile_pool(name="singles", bufs=1))
        xt_pool = ctx.enter_context(tc.tile_pool(name="xt", bufs=1))
        tab_pool = ctx.enter_context(tc.tile_pool(name="tabs", bufs=2))
        u_pool = ctx.enter_context(tc.tile_pool(name="u", bufs=2))
        work = ctx.enter_context(tc.tile_pool(name="work", bufs=2))
        opool = ctx.enter_context(tc.tile_pool(name="opool", bufs=2))
        gpool = ctx.enter_context(tc.tile_pool(name="gpool", bufs=3))
        rpool = ctx.enter_context(tc.tile_pool(name="res", bufs=3))
        ps_ab = ctx.enter_context(tc.tile_pool(name="ps_ab", bufs=1, space="PSUM"))
        ps_c = ctx.enter_context(tc.tile_pool(name="ps_c", bufs=4, space="PSUM"))
        ps_d = ctx.enter_context(tc.tile_pool(name="ps_d", bufs=2, space="PSUM"))

        # ---- weights: wall_a (pa/pc) on the SP queue, wall_b (pb) on the
        # scalar queue, per-fg chunks interleaved with the first x chunks so
        # the first matmuls (subtile deps) start as early as possible
        wall_a = singles.tile([128, FG, 2, H], F16)   # [., fg, 0,*]=btr2 [.,fg,1,*]=btr
        wall_b = singles.tile([128, FG, 2, H], F16)
        wa_r = wa_d.rearrange("(fg p) t h -> p fg t h", p=128)
        wb_r = wb_d.rearrange("(fg p) t h -> p fg t h", p=128)

        # ---- x streams in half-J chunks (2KB dram rows for fast DMA):
        # xe halves on the SP queue, xo halves on the scalar queue
        xte = [[xt_pool.tile([128, FG, 1024], F16, tag=f"xte{b}_{h}",
                             name=f"xte{b}_{h}") for h in range(2)]
               for b in range(B_LOC)]
        xto = [[xt_pool.tile([128, FG, 1040], F16, tag=f"xto{b}_{h}",
                             name=f"xto{b}_{h}") for h in range(2)]
               for b in range(B_LOC)]

        def load_xe_half(b, h, fg=None):
            xe_r = xe_d[b].rearrange("(fg p) j -> p fg j", p=128)
            jsl = slice(1024 * h, 1024 * h + 1024)
            if fg is None:
                nc.sync.dma_start(out=xte[b][h][:, :, :], in_=xe_r[:, :, jsl])
            else:
                nc.sync.dma_start(out=xte[b][h][:, fg, :],
                                  in_=xe_r[:, fg, jsl])

        def load_xo_half(b, h, fgs=None):
            xo_r = xo_d[b].rearrange("(fg p) j -> p fg j", p=128)
            jsl = slice(1024 * h, 1024 * h + 1040)
            if fgs is None:
                nc.scalar.dma_start(out=xto[b][h][:, :, :], in_=xo_r[:, :, jsl])
            else:
                nc.scalar.dma_start(out=xto[b][h][:, fgs, :],
                                    in_=xo_r[:, fgs, jsl])

        # interleave weight-fg and x-fg chunks on both queues
        for fg in range(FG):
            nc.sync.dma_start(out=wall_a[:, fg], in_=wa_r[:, fg])
            load_xe_half(0, 0, fg)
        load_xe_half(0, 1)

        # ---- small vectors + identity (tiny; on sync between xe halves)
        r2_s = singles.tile([128, HG], FP32)
        gr0_s = singles.tile([128, HG], FP32)
        gi0_s = singles.tile([128, HG], FP32)
        nc.sync.dma_start(out=r2_s, in_=r2_d.rearrange("(hg p) -> p hg", p=128))
        nc.sync.dma_start(out=gr0_s, in_=gr0_d.rearrange("(hg p) -> p hg", p=128))
        nc.sync.dma_start(out=gi0_s, in_=gi0_d.rearrange("(hg p) -> p hg", p=128))
        from concourse.masks import make_identity
        ident = singles.tile([128, 128], F16)
        make_identity(nc, ident)
        nident = singles.tile([128, 128], F16)
        nc.vector.tensor_scalar(nident, ident, -1.0, None, op0=A.mult)

        r2bc = singles.tile([128, HG, W], FP32)
        nc.vector.memset(r2bc, 1.0)
        for hg in range(HG):
            nc.vector.tensor_scalar(r2bc[:, hg, :], r2bc[:, hg, :],
                                    r2_s[:, hg:hg + 1], None, op0=A.mult)

        # ---- tables: per-hg single dispatches on the scalar queue
        cs_t = {}
        ps_t = {}

        def alloc_tabs(hg):
            cs_t[hg] = tab_pool.tile([128, 2, J], F16, tag="cs",
                                     name=f"cs{hg}")
            ps_t[hg] = tab_pool.tile([128, 2, J], F16, tag="ps",
                                     name=f"ps{hg}")

        def load_cs_half(hg, half):
            jsl = slice(half * J // 2, (half + 1) * J // 2)
            nc.scalar.dma_start(out=cs_t[hg][:, :, jsl], in_=cs_d[hg][:, :, jsl])

        def load_ps_half(hg, half):
            jsl = slice(half * J // 2, (half + 1) * J // 2)
            nc.scalar.dma_start(out=ps_t[hg][:, :, jsl], in_=ps_d_t[hg][:, :, jsl])

        # scalar queue up-front: walls + b0 xo + hg0/hg1 first-half tables.
        # Later loads are issued inside the unit loop so their dispatch
        # instructions don't clog the scalar engine ahead of PSUM staging.
        alloc_tabs(0)
        alloc_tabs(1)
        nc.scalar.dma_start(out=wall_b[:, 0:2], in_=wb_r[:, 0:2])
        load_xo_half(0, 0, slice(0, 2))
        nc.scalar.dma_start(out=wall_b[:, 2:4], in_=wb_r[:, 2:4])
        load_xo_half(0, 0, slice(2, 4))
        load_cs_half(0, 0)
        load_xo_half(0, 1)
        load_ps_half(0, 0)
        load_cs_half(1, 0)
        load_ps_half(1, 0)

        # ---- work unit list: (hg, b, j0, w). hg0/hg1 interleaved first so
        # early GEMMs need only early x quarters; final stream tapered.
        units = [
            (0, 0, 0, 1024), (1, 0, 0, 1024), (0, 0, 1024, 1024), (1, 0, 1024, 1024),
            (0, 1, 0, 1024), (1, 1, 0, 1024), (0, 1, 1024, 1024), (1, 1, 1024, 1024),
            (2, 0, 0, 1024), (3, 0, 0, 1024), (2, 0, 1024, 1024), (3, 0, 1024, 1024),
            (2, 1, 0, 1024), (3, 1, 0, 1024), (2, 1, 1024, 1024),
            (3, 1, 1024, 512), (3, 1, 1536, 512),
        ]

        pending = []

        def flush_one(p):
            for pc, ssl, sw in p["pcs"]:
                nc.tensor.matmul(pc[:, :sw], ident, p["o3"][:, ssl],
                                 start=False, stop=False)
                nc.tensor.matmul(pc[:, :sw], nident, p["o4"][:, ssl],
                                 start=False, stop=True)
                nc.scalar.copy(out=p["res_o"][:, ssl], in_=pc[:, :sw])
                pd = ps_d.tile([128, PW], FP32, tag="pd")
                nc.tensor.matmul(pd[:, :sw], ident, p["o1"][:, ssl],
                                 start=True, stop=False)
                nc.tensor.matmul(pd[:, :sw], nident, p["o2"][:, ssl],
                                 start=False, stop=True)
                nc.scalar.copy(out=p["res_e"][:, ssl], in_=pd[:, :sw])
            nc.sync.dma_start(out=oute_d[p["b"], p["hsl"], p["jsl"]],
                              in_=p["res_e"][:, :p["w"]])
            nc.scalar.dma_start(out=outo_d[p["b"], p["hsl"], p["jsl"]],
                                in_=p["res_o"][:, :p["w"]])

        gprev = {}

        def late_loads(ui):
            if ui == 1:
                load_cs_half(0, 1)
                load_ps_half(0, 1)
            elif ui == 2:
                load_xo_half(1, 0)
                load_xo_half(1, 1)
                load_xe_half(1, 0)
                load_xe_half(1, 1)
            elif ui == 3:
                load_cs_half(1, 1)
                load_ps_half(1, 1)
            elif ui == 5:
                alloc_tabs(2)
                load_cs_half(2, 0)
                load_ps_half(2, 0)
            elif ui == 7:
                alloc_tabs(3)
                load_cs_half(3, 0)
                load_ps_half(3, 0)
            elif ui == 9:
                load_cs_half(2, 1)
                load_ps_half(2, 1)
            elif ui == 11:
                load_cs_half(3, 1)
                load_ps_half(3, 1)

        for ui, (hg, b, j0, w) in enumerate(units):
            late_loads(ui)
            hsl = slice(hg * 128, (hg + 1) * 128)
            if len(pending) >= 2:
                flush_one(pending.pop(0))
            ur_sb = u_pool.tile([128, W], F16, tag="ur_sb")
            ui_sb = u_pool.tile([128, W], F16, tag="ui_sb")
            pcs = []
            nps = (w + PW - 1) // PW
            for ps in range(nps):
                p0 = j0 + ps * PW
                sw = min(PW, w - ps * PW)
                xh = p0 // 1024               # x half tile
                lo = p0 - 1024 * xh           # offset within half
                osl = slice(lo, lo + sw)
                osl1 = slice(lo + 1, lo + sw + 1)
                ssl = slice(ps * PW, ps * PW + sw)
                pa = ps_ab.tile([128, PW], FP32, tag="pa")
                pb = ps_ab.tile([128, PW], FP32, tag="pb")
                pc = ps_c.tile([128, PW], FP32, tag="pc")
                for fg in range(FG):
                    nc.tensor.matmul(pa[:, :sw], wall_a[:, fg, 0, hsl],
                                     xto[b][xh][:, fg, osl],
                                     start=(fg == 0), stop=False)
                for fg in range(FG):
                    nc.tensor.matmul(pa[:, :sw], wall_a[:, fg, 1, hsl],
                                     xte[b][xh][:, fg, osl],
                                     start=False, stop=(fg == FG - 1))
                for fg in range(FG):
                    nc.tensor.matmul(pb[:, :sw], wall_b[:, fg, 0, hsl],
                                     xto[b][xh][:, fg, osl],
                                     start=(fg == 0), stop=False)
                for fg in range(FG):
                    nc.tensor.matmul(pb[:, :sw], wall_b[:, fg, 1, hsl],
                                     xte[b][xh][:, fg, osl],
                                     start=False, stop=(fg == FG - 1))
                for fg in range(FG):
                    nc.tensor.matmul(pc[:, :sw], wall_a[:, fg, 1, hsl],
                                     xto[b][xh][:, fg, osl1],
                                     start=(fg == 0), stop=False)
                nc.scalar.copy(out=ur_sb[:, ssl], in_=pa[:, :sw])
                nc.scalar.copy(out=ui_sb[:, ssl], in_=pb[:, :sw])
                pcs.append((pc, ssl, sw))

            jsl = slice(j0, j0 + w)
            cw = cs_t[hg][:, 0, jsl]
            sw = cs_t[hg][:, 1, jsl]
            # input rotation: v = e^{-i*2theta(j+1)} u~
            t1 = work.tile([128, W], F16, tag="t1")
            t2 = work.tile([128, W], F16, tag="t2")
            t3 = work.tile([128, W], F16, tag="t3")
            t4 = work.tile([128, W], F16, tag="t4")
            vr = work.tile([128, W], F16, tag="vr")
            vi = work.tile([128, W], F16, tag="vi")
            nc.vector.tensor_mul(t1[:, :w], cw, ur_sb[:, :w])
            nc.vector.tensor_mul(t2[:, :w], sw, ui_sb[:, :w])
            nc.vector.tensor_add(vr[:, :w], t1[:, :w], t2[:, :w])
            nc.vector.tensor_mul(t3[:, :w], cw, ui_sb[:, :w])
            nc.vector.tensor_mul(t4[:, :w], sw, ur_sb[:, :w])
            nc.vector.tensor_sub(vi[:, :w], t3[:, :w], t4[:, :w])

            gr = gpool.tile([128, W], F16, tag="gr")
            gi = gpool.tile([128, W], F16, tag="gi")
            if j0 == 0:
                init_r = gr0_s[:, hg:hg + 1]
                init_i = gi0_s[:, hg:hg + 1]
            else:
                gr_p, gi_p, wp = gprev[(hg, b)]
                init_r = gr_p[:, wp - 1:wp]
                init_i = gi_p[:, wp - 1:wp]
            nc.vector.tensor_tensor_scan(gr[:, :w], r2bc[:, hg, :w],
                                         vr[:, :w], init_r,
                                         op0=A.mult, op1=A.add)
            nc.vector.tensor_tensor_scan(gi[:, :w], r2bc[:, hg, :w],
                                         vi[:, :w], init_i,
                                         op0=A.mult, op1=A.add)
            gprev[(hg, b)] = (gr, gi, w)

            # output: even t=2j -> C.Gr - S.Gi ; odd -> P1.Gr - P2.Gi + uo
            o1 = opool.tile([128, W], F16, tag="o1")
            o2 = opool.tile([128, W], F16, tag="o2")
            o3 = opool.tile([128, W], F16, tag="o3")
            o4 = opool.tile([128, W], F16, tag="o4")
            res_e = rpool.tile([128, W], F16, tag="res_e")
            res_o = rpool.tile([128, W], F16, tag="res_o")
            nc.vector.tensor_mul(o1[:, :w], cw, gr[:, :w])
            nc.vector.tensor_mul(o2[:, :w], sw, gi[:, :w])
            nc.vector.tensor_mul(o3[:, :w], ps_t[hg][:, 0, jsl], gr[:, :w])
            nc.vector.tensor_mul(o4[:, :w], ps_t[hg][:, 1, jsl], gi[:, :w])
            pending.append(dict(pcs=pcs, o1=o1, o2=o2, o3=o3, o4=o4,
                                res_e=res_e, res_o=res_o,
                                b=b, hsl=hsl, jsl=jsl, w=w))

        while pending:
            flush_one(pending.pop(0))

    nc.compile()
    return nc


def _prepare(inputs):
    x = np.asarray(inputs["x"], dtype=np.float32)
    B_real = np.asarray(inputs["B_real"], dtype=np.float64)
    B_img = np.asarray(inputs["B_img"], dtype=np.float64)
    nu = np.asarray(inputs["nu"], dtype=np.float64)
    theta = np.asarray(inputs["theta"], dtype=np.float64)
    delta = np.asarray(inputs["delta"], dtype=np.float64)
    h0r = np.asarray(inputs["h0_real"], dtype=np.float64)
    h0i = np.asarray(inputs["h0_img"], dtype=np.float64)

    r = np.exp(-np.exp(nu))
    btr = B_real * delta[None, :]
    bti = B_img * delta[None, :]
    rc = r * np.cos(theta)
    rs = r * np.sin(theta)
    btr2 = btr * rc[None, :] - bti * rs[None, :]
    bti2 = btr * rs[None, :] + bti * rc[None, :]

    jj = np.arange(J, dtype=np.float64)
    ang_c = theta[:, None] * (2.0 * jj + 2.0)[None, :]   # 2theta(j+1)
    ctab = np.cos(ang_c)
    stab = np.sin(ang_c)
    ang_p = theta[:, None] * (2.0 * jj + 3.0)[None, :]   # theta(2j+3)
    p1 = r[:, None] * np.cos(ang_p)
    p2 = r[:, None] * np.sin(ang_p)

    # G_{-1} = h0 / lam = h0 * e^{-i theta} / r
    ginr = (h0r * np.cos(theta) + h0i * np.sin(theta)) / r
    gini = (h0i * np.cos(theta) - h0r * np.sin(theta)) / r

    xh = x.astype(np.float16)
    # host-side transpose to [B, F, J] so device loads are contiguous
    xe = np.ascontiguousarray(xh[:, 0::2, :].transpose(0, 2, 1))   # (B, F, J)
    xo = np.zeros((B, F, JOP), dtype=np.float16)
    xo[:, :, 1:J + 1] = xh[:, 1::2, :].transpose(0, 2, 1)          # slot p = x_{2p-1}

    wall_a = np.stack([btr2, btr], axis=1).astype(np.float16)      # (F, 2, H)
    wall_b = np.stack([bti2, bti], axis=1).astype(np.float16)
    cth = ctab.reshape(HG, 128, J)
    sth = stab.reshape(HG, 128, J)
    cs_tab = np.stack([cth, sth], axis=2).astype(np.float16)       # (HG,128,2,J)
    p1h = p1.reshape(HG, 128, J)
    p2h = p2.reshape(HG, 128, J)
    ps_tab = np.stack([p1h, p2h], axis=2).astype(np.float16)

    return dict(
        wall_a=wall_a, wall_b=wall_b,
        r2vec=(r * r).astype(np.float32),
        cs_tab=cs_tab, ps_tab=ps_tab,
        ginr=ginr.astype(np.float32), gini=gini.astype(np.float32),
    ), xe, xo


_NC_CACHE = {}


def get_program():
    if "nc" not in _NC_CACHE:
        _NC_CACHE["nc"] = build_program()
    return _NC_CACHE["nc"]


def make_in_maps(inputs):
    shared, xe, xo = _prepare(inputs)
    return [dict(xe=np.ascontiguousarray(xe[c * B_LOC:(c + 1) * B_LOC]),
                 xo=np.ascontiguousarray(xo[c * B_LOC:(c + 1) * B_LOC]),
                 **shared)
            for c in range(N_CORES)]


def kernel(**inputs) -> np.ndarray:
    from concourse.bass_utils import run_bass_kernel_spmd

    nc = get_program()
    in_maps = make_in_maps(inputs)
    res = run_bass_kernel_spmd(nc, in_maps, list(range(N_CORES)))
    out = np.empty((B, L, H), dtype=np.float32)
    for c in range(N_CORES):
        sl = slice(c * B_LOC, (c + 1) * B_LOC)
        out[sl, 0::2, :] = res.results[c]["oute"].transpose(0, 2, 1)
        out[sl, 1::2, :] = res.results[c]["outo"].transpose(0, 2, 1)
    return out
